# revision 1
# baseline (speedup 1.0000x reference)
"""Lensiformer forward pass on 8 Trainium2 NeuronCores.

Strategy: data-parallel over batch (32 images -> 4 per core, params
replicated, no collectives). Per core, a single fused Bass/Tile program
runs the whole network:

  - patch embed as matmul over host-im2col'd patches (conv == matmul),
    both shifted-patch tokenizers share one matmul (same input image)
  - gate/fuse, cls+pos assembly directly in transformer token layout
  - 8 transformer layers, fp32 throughout:
      tokens padded 1028 -> 1152 (9 tiles of 128) for dense phases
      LN via bn_stats; LN gains/biases folded into the following
      matmul weights on host (exact algebraic rewrite)
      QKV: Q^T/K^T computed feature-major per image, V token-major
      attention computed transposed (scores^T) so softmax needs no
      P-transposes; no max-subtraction (scores are O(1) here; exp is
      safe in fp32); denominator via an all-ones 65th column appended
      to each head's V block; 1/denom applied to O^T via a tiny
      broadcast matmul before proj
      MLP hidden computed feature-major (gelu+bias fused into the
      PSUM->SBUF activation copy), consumed directly as mlp2 lhsT
  - final LN + 3-way head on the 4 cls tokens

Self-contained: includes the walrus sync-wait-limit workaround and the
axon NTFF profiling shim.
"""
import contextlib
import ctypes
import sys
import types

import numpy as np

import concourse.bass as bass
import concourse.mybir as mybir
import concourse.tile as tile
from concourse.masks import make_identity
from concourse.vector_clock import ScopedClock

F32 = mybir.dt.float32
F32R = mybir.dt.float32r
AF = mybir.ActivationFunctionType
ALU = mybir.AluOpType

# ---------------- model geometry (hardcoded from the problem spec) ----------
B, IMG, PATCH = 32, 128, 8
D, H, L, MLP, NCLS = 512, 8, 8, 2048, 3
GRID = IMG // PATCH            # 16
P = GRID * GRID                # 256 patches / image
N = P + 1                      # 257 tokens / image
HD = D // H                    # 64
KC = 320                       # im2col contraction: 5 shifts * 8 * 8
NCORES = 8
NIMG = B // NCORES             # 4 images / core
TP = NIMG * P                  # 1024 patch tokens / core
NT = NIMG * N                  # 1028 transformer tokens / core
NTILE = 9                      # token tiles of 128
TT = NTILE * 128               # 1152 padded tokens
IMGOFF = [i * N for i in range(NIMG)]
NEG = -1.0e30
USE_F32R = True
NQ = 258
NQ2 = NQ - 256

_PROGRAM_CACHE = {}

# ============================================================================
# environment fixups
# ============================================================================
_fixups_done = False


def _install_fixups():
    global _fixups_done
    if _fixups_done:
        return
    _fixups_done = True
    MAXW = 1

    def _split_waits(nc, ordered):
        for bb_name, insts in ordered.items():
            new_list = []
            for inst in insts:
                si = getattr(inst, 'sync_info', None)
                eng = getattr(inst, 'engine', None)
                if (si is not None and si.on_wait and len(si.on_wait) > MAXW
                        and eng is not None
                        and type(inst).__name__.startswith('Inst')):
                    waits = list(si.on_wait)
                    inst.sync_info = mybir.SyncInfo(
                        on_wait=waits[:MAXW], on_update=list(si.on_update or []))
                    for i in range(MAXW, len(waits), MAXW):
                        new_list.append(mybir.InstNoOp(
                            name=nc.get_next_instruction_name(),
                            engine=eng, bass_nofuse=True,
                            sync_info=mybir.SyncInfo(
                                on_wait=waits[i:i + MAXW], on_update=[])))
                new_list.append(inst)
            ordered[bb_name] = new_list

    orig_lower = tile.TileContext._lower_ordered_insts

    def patched_lower(self, ordered):
        _split_waits(self.nc, ordered)
        return orig_lower(self, ordered)

    tile.TileContext._lower_ordered_insts = patched_lower

    def patched_drain_and_barrier(self, tick_clock, wait_clock):
        drain_inst = self.nc.sync.drain()
        wait_clock.add_sem_waits(
            drain_inst.ins, ScopedClock({None: tick_clock.global_clock}))
        si = drain_inst.ins.sync_info
        if si and si.on_wait and len(si.on_wait) > MAXW:
            waits = list(si.on_wait)
            drain_inst.ins.sync_info = mybir.SyncInfo(
                on_wait=waits[:MAXW], on_update=list(si.on_update or []))
            for i in range(MAXW, len(waits), MAXW):
                extra = self.nc.sync.drain()
                extra.ins.sync_info = mybir.SyncInfo(
                    on_wait=waits[i:i + MAXW], on_update=[])
        self.nc.all_engine_barrier()
        assert self.sems is not None
        popped = self.nc._tile_sem_poison_stack.pop()
        assert popped is self._sem_poison
        self.nc.clear_and_free_semaphores(list(self.sems.allocated().values()))
        self.nc.all_engine_barrier()

    tile.TileContext._drain_and_barrier = patched_drain_and_barrier

    if 'antenv.axon_hooks' not in sys.modules:
        holder = {'h': None}
        mod = types.ModuleType('antenv.axon_hooks')
        mod.set_axon_ntff_profile_hook = lambda h: holder.__setitem__('h', h)
        mod.get_axon_ntff_profile_hook = lambda: holder['h']
        sys.modules['antenv.axon_hooks'] = mod
        try:
            lib = ctypes.CDLL('/opt/axon/libaxon_pjrt.so')
            if hasattr(lib, 'axon_start_nrt_profile'):
                lib.axon_start_nrt_profile.argtypes = [
                    ctypes.POINTER(ctypes.c_int64), ctypes.c_size_t]
                lib.axon_start_nrt_profile.restype = ctypes.c_int64
                lib.axon_stop_nrt_profile.argtypes = [ctypes.c_char_p]
                lib.axon_stop_nrt_profile.restype = ctypes.c_int64

                @contextlib.contextmanager
                def _hook(output_dir, device_ids):
                    import jax
                    jax.devices()
                    if device_ids:
                        ids = (ctypes.c_int64 * len(device_ids))(*device_ids)
                        rc = lib.axon_start_nrt_profile(ids, len(device_ids))
                    else:
                        rc = lib.axon_start_nrt_profile(None, 0)
                    if rc != 0:
                        raise RuntimeError(f'axon_start_nrt_profile rc={rc}')
                    try:
                        yield
                    finally:
                        lib.axon_stop_nrt_profile(output_dir.encode())

                mod.set_axon_ntff_profile_hook(_hook)
        except OSError:
            pass


# ============================================================================
# host-side input marshaling (pure data movement + tiny param folds)
# ============================================================================
def _im2col(image):
    """(Bc,1,IMG,IMG) -> (Bc, P, 320), col order [shift, py, px]."""
    shifts = [(0, 0), (1, 1), (-1, 1), (1, -1), (-1, -1)]
    x = image[:, 0]
    cols = []
    for (sy, sx) in shifts:
        xs = np.roll(x, (sy, sx), (1, 2))
        pt = xs.reshape(-1, GRID, PATCH, GRID, PATCH).transpose(0, 1, 3, 2, 4)
        cols.append(pt.reshape(-1, P, PATCH * PATCH))
    return np.concatenate(cols, -1)


def _rne12(a):
    """Round fp32 array to f32r (RNE at 12 low mantissa bits) - matches HW."""
    bits = np.ascontiguousarray(a, np.float32).view(np.uint32)
    half = np.uint32(1 << 11)
    mask = np.uint32((1 << 12) - 1)
    low = bits & mask
    up = (low > half) | ((low == half) & ((bits >> 12) & 1).astype(bool))
    out = ((bits & ~mask) + np.where(up, np.uint32(1 << 12), np.uint32(0)))
    return out.view(np.float32)


def _host_prep(inputs):
    f = lambda k: np.ascontiguousarray(np.asarray(inputs[k], np.float32))
    image = f('image')

    # conv weights -> matmul form, both tokenizers side by side
    wconv = np.concatenate(
        [f('ssw').reshape(D, KC).T, f('sow').reshape(D, KC).T], 1)  # (320,1024)
    bconv = np.concatenate([f('ssb'), f('sob')])                    # (1024,)
    gbeta = np.stack([np.concatenate([f('ssg'), f('sog')]),
                      np.concatenate([f('ssbeta'), f('sobeta')])])  # (2,1024)

    # fold LN gains/biases into the following matmuls (exact rewrite)
    ln1g, ln1b = f('ln1g'), f('ln1b')
    ln2g, ln2b = f('ln2g'), f('ln2b')
    qkvw, qkvb = f('qkvw'), f('qkvb')
    w1, b1 = f('w1'), f('b1')
    qkvw_eff = ln1g[:, :, None] * qkvw
    qkvb_eff = qkvb + np.einsum('ld,ldn->ln', ln1b, qkvw)
    w1_eff = ln2g[:, :, None] * w1
    b1_eff = b1 + np.einsum('ld,ldn->ln', ln2b, w1)
    hw_eff = f('ng')[:, None] * f('hw')
    hb_eff = f('hb') + f('nb') @ f('hw')

    # pos/cls in padded transformer layout
    pos = f('pos_embed')[0]          # (257, 512)
    cls_eff = f('cls_token')[0, 0] + pos[0]
    pospad = np.zeros((TT, D), np.float32)
    for i in range(NIMG):
        pospad[IMGOFF[i]] = cls_eff
        pospad[IMGOFF[i] + 1: IMGOFF[i] + N] = pos[1:]

    X = _im2col(image)               # (B, P, 320)

    rk = _rne12 if USE_F32R else (lambda x: x)
    common = dict(
        wconv=rk(wconv), bconv=rk(bconv), gbeta=gbeta,
        fw=rk(f('fw')), fb=rk(f('fb')), pospad=pospad,
        qkvw=rk(qkvw_eff), qkvb=rk(qkvb_eff),
        projw=rk(f('projw')), projb=rk(f('projb')), temp=f('temp'),
        w1=rk(w1_eff), b1=np.ascontiguousarray(b1_eff),
        w2=rk(f('w2')), b2=rk(f('b2')),
        hw=rk(np.concatenate([hw_eff, np.zeros((D, 1), np.float32)], 1)),
        hb=rk(np.concatenate([f('hb') + f('nb') @ f('hw'),
                              np.zeros(1, np.float32)])),
    )
    in_maps = []
    for c in range(NCORES):
        xt = rk(np.ascontiguousarray(
            X[c * NIMG:(c + 1) * NIMG].reshape(TP, KC).T))  # (320, 1024)
        m = dict(common)
        m['xt'] = xt
        in_maps.append(m)
    return in_maps


# ============================================================================
# device program
# ============================================================================
def _tile_segments(t):
    """Real-token segments of token-tile t: (row_in_tile, n, img, pos0)."""
    segs = []
    r0 = 128 * t
    for img in range(NIMG):
        lo = max(r0, IMGOFF[img])
        hi = min(r0 + 128, IMGOFF[img] + N, NT)
        if lo < hi:
            segs.append((lo - r0, hi - lo, img, lo - IMGOFF[img]))
    return segs


def _build_program():
    nc = bass.Bass()

    MMDT = F32R if USE_F32R else F32
    din = lambda nm, sh, dt_=F32: nc.dram_tensor(nm, sh, dt_, kind='ExternalInput')
    xt_d = din('xt', [KC, TP], MMDT)
    wc_d = din('wconv', [KC, 2 * D], MMDT)
    bc_d = din('bconv', [2 * D], MMDT)
    gb_d = din('gbeta', [2, 2 * D])
    fw_d = din('fw', [2 * D, D], MMDT)
    fb_d = din('fb', [D], MMDT)
    pos_d = din('pospad', [TT, D])
    qkvw_d = din('qkvw', [L, D, 3 * D], MMDT)
    qkvb_d = din('qkvb', [L, 3 * D], MMDT)
    projw_d = din('projw', [L, D, D], MMDT)
    projb_d = din('projb', [L, D], MMDT)
    temp_d = din('temp', [L, H])
    w1_d = din('w1', [L, D, MLP], MMDT)
    b1_d = din('b1', [L, MLP])
    w2_d = din('w2', [L, MLP, D], MMDT)
    b2_d = din('b2', [L, D], MMDT)
    hw_d = din('hw', [D, 4], MMDT)
    hb_d = din('hb', [4], MMDT)
    out_d = nc.dram_tensor('out', [NIMG, 4], F32, kind='ExternalOutput')

    with tile.TileContext(nc) as tc, \
            nc.allow_low_precision(reason='f32r matmul input rounding'):
        with contextlib.ExitStack() as ctx:
            sb = ctx.enter_context(tc.tile_pool(name='sb', bufs=1))
            ps = ctx.enter_context(tc.tile_pool(name='ps', bufs=7, space='PSUM'))

            _psn = [0]

            def pstile():
                _psn[0] += 1
                return ps.tile([128, 512], F32, tag='ps', bufs=7,
                               name=f'ps{_psn[0]}')


            # ---------------- constants ----------------
            ident = sb.tile([128, 128], F32, tag='ident')
            make_identity(nc, ident)
            negdiag = sb.tile([128, 128], F32, tag='negdiag')
            nc.scalar.mul(out=negdiag, in_=ident, mul=NEG)
            onesf = sb.tile([1, 128], F32, tag='onesf')
            nc.vector.memset(onesf, 1.0)
            ones128 = sb.tile([128, 32], F32, tag='ones128')
            nc.vector.memset(ones128, 1.0)
            zeros = sb.tile([128, 544], F32, tag='zeros')
            nc.vector.memset(zeros, 0.0)
            negc2 = sb.tile([NQ2, NQ], F32, tag='negc2')
            nc.vector.memset(negc2, NEG)
            nc.vector.memset(negc2[0:1, :], 0.0)
            nc.vector.memset(negc2[0:1, 256:257], NEG)
            ones_row = sb.tile([1, 128], MMDT, tag='ones_row')
            nc.vector.tensor_copy(out=ones_row, in_=onesf)
            eps = sb.tile([128, 1], F32, tag='eps')
            nc.vector.memset(eps, 1e-5)

            # ---------------- persistent activations ----------------
            tok = sb.tile([128, NTILE, D], F32, tag='tok')       # residual
            xT = None                                            # per phase
            ot = None

            # LN helper: writes normalized tile via ACT, returns nothing
            def layer_norm_apply(src_ap, dst_ap, n_rows=128):
                stats = sb.tile([128, 6], F32, tag='lnstat', bufs=4)
                mv = sb.tile([128, 2], F32, tag='lnmv', bufs=4)
                nc.vector.bn_stats(out=stats[0:n_rows], in_=src_ap)
                nc.vector.bn_aggr(out=mv[0:n_rows], in_=stats[0:n_rows])
                std = sb.tile([128, 1], F32, tag='lnstd', bufs=4)
                nc.scalar.activation(out=std[0:n_rows], in_=mv[0:n_rows, 1:2],
                                     func=AF.Sqrt, bias=eps[0:n_rows], scale=1.0)
                rstd = sb.tile([128, 1], F32, tag='lnrstd', bufs=4)
                nc.vector.reciprocal(out=rstd[0:n_rows], in_=std[0:n_rows])
                nmr = sb.tile([128, 1], F32, tag='lnnmr', bufs=4)
                nc.vector.scalar_tensor_tensor(
                    out=nmr[0:n_rows], in0=mv[0:n_rows, 0:1], scalar=-1.0,
                    in1=rstd[0:n_rows], op0=ALU.mult, op1=ALU.mult)
                nc.scalar.activation(out=dst_ap, in_=src_ap, func=AF.Identity,
                                     scale=rstd[0:n_rows], bias=nmr[0:n_rows])

            # ================= patch embed =================
            with tc.tile_pool(name='emb', bufs=1) as emb:
                xt_sb = []
                for kc, k0, kn in ((0, 0, 128), (1, 128, 128), (2, 256, 64)):
                    t_ = emb.tile([kn, TP], MMDT, tag=f'xt{kc}')
                    nc.sync.dma_start(out=t_, in_=xt_d[k0:k0 + kn, :])
                    xt_sb.append(t_)
                wc_sb = []
                for kc, k0, kn in ((0, 0, 128), (1, 128, 128), (2, 256, 64)):
                    t_ = emb.tile([kn, 2 * D], MMDT, tag=f'wc{kc}')
                    nc.sync.dma_start(out=t_, in_=wc_d[k0:k0 + kn, :])
                    wc_sb.append(t_)
                bc_sb = emb.tile([1, 2 * D], MMDT, tag='bc')
                nc.sync.dma_start(out=bc_sb, in_=bc_d[None, :])
                gb_g = emb.tile([128, 2 * D], F32, tag='gbg')
                nc.sync.dma_start(
                    out=gb_g, in_=gb_d[0][None, :].to_broadcast([128, 2 * D]))
                gb_b = emb.tile([128, 2 * D], F32, tag='gbb')
                nc.sync.dma_start(
                    out=gb_b, in_=gb_d[1][None, :].to_broadcast([128, 2 * D]))
                fw_sb = emb.tile([128, 8, D], MMDT, tag='fwsb')
                nc.sync.dma_start(
                    out=fw_sb, in_=fw_d[:, :].rearrange('(c p) n -> p c n', p=128))
                fb_sb = emb.tile([1, D], MMDT, tag='fbsb')
                nc.sync.dma_start(out=fb_sb, in_=fb_d[None, :])

                fused_d = nc.dram_tensor('fusedbuf', [TP, D], F32)
                for t in range(TP // 128):      # 8 patch-layout tiles
                    combraw = emb.tile([128, 2 * D], F32, tag='combraw', bufs=2)
                    for nh in range(2):
                        cps = pstile()
                        nc.tensor.matmul(cps, ones_row[0:1, :],
                                         bc_sb[0:1, nh * D:(nh + 1) * D],
                                         start=True, stop=False,
                                         skip_group_check=True)
                        for kc in range(3):
                            nc.tensor.matmul(
                                cps, xt_sb[kc][:, t * 128:(t + 1) * 128],
                                wc_sb[kc][:, nh * D:(nh + 1) * D],
                                start=False, stop=(kc == 2),
                                skip_group_check=True)
                        nc.vector.tensor_copy(
                            out=combraw[:, nh * D:(nh + 1) * D], in_=cps)

                    comb = emb.tile([128, 2 * D], F32, tag='comb', bufs=2)
                    layer_norm_apply(combraw[:, 0:D], comb[:, 0:D])
                    layer_norm_apply(combraw[:, D:2 * D], comb[:, D:2 * D])
                    nc.vector.tensor_mul(comb, comb, gb_g)
                    nc.vector.tensor_add(comb, comb, gb_b)

                    combT = emb.tile([128, 8, 128], MMDT, tag='combT', bufs=2)
                    for c in range(8):
                        tp = pstile()
                        nc.tensor.transpose(tp[:, 0:128],
                                            comb[:, c * 128:(c + 1) * 128], ident)
                        nc.vector.tensor_copy(out=combT[:, c, :], in_=tp[:, 0:128])

                    gps = pstile()
                    nc.tensor.matmul(gps, ones_row[0:1, :], fb_sb,
                                     start=True, stop=False, skip_group_check=True)
                    for c in range(8):
                        nc.tensor.matmul(gps, combT[:, c, :], fw_sb[:, c, :],
                                         start=False, stop=(c == 7),
                                         skip_group_check=True)
                    gt = emb.tile([128, D], F32, tag='gt', bufs=2)
                    nc.scalar.activation(out=gt, in_=gps, func=AF.Sigmoid)
                    diff = emb.tile([128, D], F32, tag='diff', bufs=2)
                    nc.vector.tensor_sub(diff, comb[:, 0:D], comb[:, D:2 * D])
                    nc.vector.tensor_mul(diff, diff, gt)
                    nc.vector.tensor_add(diff, diff, comb[:, D:2 * D])
                    nc.sync.dma_start(out=fused_d[t * 128:(t + 1) * 128, :],
                                      in_=diff)

                # reshuffle patch-layout fused tokens into transformer layout,
                # zero the cls rows (pos add below then yields cls_eff there)
                nc.vector.memset(tok[:, NTILE - 1, :], 0.0)
                for t in range(NTILE):
                    for (rs, nr, img, pos0) in _tile_segments(t):
                        if pos0 == 0:
                            nc.sync.dma_start(out=tok[rs:rs + 1, t, :],
                                              in_=pos_d[TT - 1:TT, :])
                            rs, nr, pos0 = rs + 1, nr - 1, 1
                        if nr <= 0:
                            continue
                        p0 = img * P + (pos0 - 1)
                        nc.sync.dma_start(out=tok[rs:rs + nr, t, :],
                                          in_=fused_d[p0:p0 + nr, :])
                    postile = emb.tile([128, D], F32, tag='pos', bufs=2)
                    nc.sync.dma_start(out=postile,
                                      in_=pos_d[t * 128:(t + 1) * 128, :])
                    nc.vector.tensor_add(tok[:, t, :], tok[:, t, :], postile)

            # ================= transformer layers =================
            lay = ctx.enter_context(tc.tile_pool(name='lay', bufs=1))
            for l in range(L):
                qkvw_sb = lay.tile([128, 4, 3 * D], MMDT, tag='wt', bufs=2)
                nc.sync.dma_start(
                    out=qkvw_sb,
                    in_=qkvw_d[l].rearrange('(c p) n -> p c n', p=128))
                qkvb_sb = lay.tile([128, 12], F32, tag='qkvb', bufs=2)
                nc.sync.dma_start(
                    out=qkvb_sb,
                    in_=qkvb_d[l].rearrange('(c p) -> p c', p=128).bitcast(F32))
                qkvbv = lay.tile([1, D], MMDT, tag='qkvbv', bufs=1)
                nc.sync.dma_start(out=qkvbv, in_=qkvb_d[l][None, 2 * D:3 * D])
                temp_sb = lay.tile([128, H], F32, tag='temp', bufs=2)
                nc.sync.dma_start(out=temp_sb,
                                  in_=temp_d[l][None, :].to_broadcast([128, H]))

                # ---- A: LN1 + transpose to feature-major ----
                xT = lay.tile([128, 4, TT], MMDT, tag='xT', bufs=1)
                for t in range(NTILE):
                    xn = lay.tile([128, D], F32, tag='xn', bufs=2)
                    layer_norm_apply(tok[:, t, :], xn)
                    for c in range(4):
                        tp = pstile()
                        nc.tensor.transpose(tp[:, 0:128],
                                            xn[:, c * 128:(c + 1) * 128], ident)
                        nc.vector.tensor_copy(
                            out=xT[:, c, t * 128:(t + 1) * 128], in_=tp[:, 0:128])

                # ---- batched V for each image's last token (row 256) ----
                vlast = lay.tile([NQ2, NIMG, H * 65], MMDT, tag='vlast', bufs=1)
                for i in range(NIMG):
                    nc.vector.tensor_copy(out=vlast[:, i, :],
                                          in_=zeros[0:NQ2, 0:H * 65])
                nc.vector.tensor_copy(
                    out=vlast.rearrange(
                        'p i (h e) -> p i h e', e=65)[0:1, :, :, 64:65],
                    in_=ones128[0:1, 0:NIMG * H])

                ot = lay.tile([128, 4, TT], MMDT, tag='ot', bufs=1)
                nc.vector.tensor_copy(out=ot[:, :, NT:TT],
                                      in_=zeros[:, 0:4 * (TT - NT)])

                for img in range(NIMG):
                    io = IMGOFF[img]
                    # ---- B_qk: Q^T,K^T feature-major for this image ----
                    qkt = lay.tile([128, 8, NQ], MMDT, tag='qkt', bufs=1)
                    for fc in range(8):
                        qps = pstile()
                        for c in range(4):
                            nc.tensor.matmul(
                                qps[:, 0:NQ],
                                qkvw_sb[:, c, fc * 128:(fc + 1) * 128],
                                xT[:, c, io:io + NQ],
                                start=(c == 0), stop=(c == 3))
                        nc.scalar.activation(
                            out=qkt[:, fc, :], in_=qps[:, 0:NQ],
                            func=AF.Identity, bias=qkvb_sb[:, fc:fc + 1],
                            scale=1.0)

                    # ---- B_v: V token-major (chunks 0,1; 2 is vlast) ----
                    vimg = lay.tile([128, 2, H * 65], MMDT, tag='vimg', bufs=1)
                    nc.vector.tensor_copy(
                        out=vimg.rearrange(
                            'p c (h e) -> p c h e', e=65)[:, :, :, 64:65],
                        in_=ones128[:, 0:2 * H])
                    for c2 in range(2):
                        vp = pstile()
                        nc.tensor.matmul(vp, ones_row[0:1, :], qkvbv,
                                         start=True, stop=False,
                                         skip_group_check=True)
                        for c in range(4):
                            nc.tensor.matmul(
                                vp, xT[:, c, io + c2 * 128:io + (c2 + 1) * 128],
                                qkvw_sb[:, c, 2 * D:3 * D],
                                start=False, stop=(c == 3),
                                skip_group_check=True)
                        nc.scalar.copy(
                            out=vimg.rearrange(
                                'p c (h e) -> p c h e', e=65)[:, c2, :, 0:64],
                            in_=vp)

                    vp2 = pstile()
                    nc.tensor.matmul(vp2[0:1, :], ones_row[0:1, 0:1], qkvbv,
                                     start=True, stop=False,
                                     skip_group_check=True)
                    for c in range(4):
                        nc.tensor.matmul(
                            vp2[0:1, :], xT[:, c, io + P:io + P + 1],
                            qkvw_sb[:, c, 2 * D:3 * D],
                            start=False, stop=(c == 3), skip_group_check=True)
                    nc.scalar.copy(
                        out=vlast.rearrange(
                            'p i (h e) -> p i h e', e=65)[0:1, img, :, 0:64],
                        in_=vp2[0:1, :])

                    # ---- C: attention (transposed softmax) ----
                    dr = lay.tile([1, 8, NQ], F32, tag='dr', bufs=1)
                    for h in range(H):
                        hr = (h % 2) * 64
                        qfc = h // 2
                        kfc = 4 + h // 2
                        pt = lay.tile([128, 3, NQ], MMDT, tag='pt', bufs=2)
                        pvp = pstile()
                        for c in range(3):
                            cm = (128, 128, NQ2)[c]
                            stp = pstile()
                            nc.tensor.matmul(
                                stp[0:cm, 0:NQ],
                                qkt[hr:hr + 64, kfc, c * 128:c * 128 + cm],
                                qkt[hr:hr + 64, qfc, :],
                                start=True, stop=True)
                            if c < 2:
                                nc.vector.tensor_add(
                                    stp[0:cm, c * 128:c * 128 + cm],
                                    stp[0:cm, c * 128:c * 128 + cm],
                                    negdiag[0:cm, 0:cm])
                            else:
                                nc.vector.tensor_add(
                                    stp[0:cm, 0:NQ], stp[0:cm, 0:NQ], negc2)
                            nc.scalar.activation(
                                out=pt[0:cm, c, :], in_=stp[0:cm, 0:NQ],
                                func=AF.Exp, scale=temp_sb[0:cm, h:h + 1])
                        for c in range(3):
                            cm = (128, 128, NQ2)[c]
                            lhs = (vimg[0:cm, c, h * 65:h * 65 + 65] if c < 2
                                   else vlast[:, img, h * 65:h * 65 + 65])
                            nc.tensor.matmul(
                                pvp[0:65, 0:NQ], lhs, pt[0:cm, c, :],
                                start=(c == 0), stop=(c == 2))
                        nc.scalar.copy(out=ot[hr:hr + 64, h // 2, io:io + NQ],
                                       in_=pvp[0:64, 0:NQ])
                        nc.vector.tensor_copy(out=dr[0:1, h, :],
                                              in_=pvp[64:65, 0:NQ])
                    rr = lay.tile([1, 8, NQ], MMDT, tag='rr', bufs=1)
                    nc.vector.reciprocal(out=rr, in_=dr)
                    for fc in range(4):
                        rtp0 = pstile()
                        nc.tensor.matmul(rtp0[0:64, 0:NQ], ones_row[0:1, 0:64],
                                         rr[0:1, 2 * fc, :],
                                         start=True, stop=True)
                        rtp1 = pstile()
                        nc.tensor.matmul(rtp1[0:64, 0:NQ], ones_row[0:1, 0:64],
                                         rr[0:1, 2 * fc + 1, :],
                                         start=True, stop=True)
                        nc.vector.tensor_mul(ot[0:64, fc, io:io + NQ],
                                             ot[0:64, fc, io:io + NQ],
                                             rtp0[0:64, 0:NQ])
                        nc.vector.tensor_mul(ot[64:128, fc, io:io + NQ],
                                             ot[64:128, fc, io:io + NQ],
                                             rtp1[0:64, 0:NQ])

                # ---- D: proj + residual ----
                projw_sb = lay.tile([128, 4, D], MMDT, tag='wt', bufs=2)
                nc.sync.dma_start(
                    out=projw_sb,
                    in_=projw_d[l].rearrange('(c p) n -> p c n', p=128))
                projb_sb = lay.tile([1, D], MMDT, tag='projb', bufs=1)
                nc.sync.dma_start(out=projb_sb, in_=projb_d[l][None, :])
                for t in range(NTILE):
                    pp = pstile()
                    nc.tensor.matmul(pp, ones_row[0:1, :], projb_sb,
                                     start=True, stop=False,
                                     skip_group_check=True)
                    for c in range(4):
                        nc.tensor.matmul(pp, ot[:, c, t * 128:(t + 1) * 128],
                                         projw_sb[:, c, :],
                                         start=False, stop=(c == 3),
                                         skip_group_check=True)
                    nc.vector.tensor_add(tok[:, t, :], tok[:, t, :], pp)

                # ---- E: LN2 + transpose ----
                xT = lay.tile([128, 4, TT], MMDT, tag='xT', bufs=1)
                for t in range(NTILE):
                    xn = lay.tile([128, D], F32, tag='xn', bufs=2)
                    layer_norm_apply(tok[:, t, :], xn)
                    for c in range(4):
                        tp = pstile()
                        nc.tensor.transpose(tp[:, 0:128],
                                            xn[:, c * 128:(c + 1) * 128], ident)
                        nc.vector.tensor_copy(
                            out=xT[:, c, t * 128:(t + 1) * 128], in_=tp[:, 0:128])

                # ---- F/G: MLP ----
                w1_sb = lay.tile([128, 4, MLP], MMDT, tag='wt', bufs=2)
                nc.sync.dma_start(
                    out=w1_sb, in_=w1_d[l].rearrange('(c p) n -> p c n', p=128))
                b1_sb = lay.tile([128, 16], F32, tag='b1', bufs=2)
                nc.sync.dma_start(
                    out=b1_sb, in_=b1_d[l].rearrange('(c p) -> p c', p=128))
                w2_sb = lay.tile([128, 16, D], MMDT, tag='wt', bufs=2)
                nc.sync.dma_start(
                    out=w2_sb, in_=w2_d[l].rearrange('(c p) n -> p c n', p=128))
                b2_sb = lay.tile([1, D], MMDT, tag='b2', bufs=1)
                nc.sync.dma_start(out=b2_sb, in_=b2_d[l][None, :])

                for g in range(3):
                    g0 = g * 384
                    gw = 384
                    hT = lay.tile([128, 16, 384], MMDT, tag='hT', bufs=1)
                    for hc in range(16):
                        hp = pstile()
                        for c in range(4):
                            nc.tensor.matmul(
                                hp[:, 0:gw],
                                w1_sb[:, c, hc * 128:(hc + 1) * 128],
                                xT[:, c, g0:g0 + gw],
                                start=(c == 0), stop=(c == 3))
                        nc.scalar.activation(
                            out=hT[:, hc, 0:gw], in_=hp[:, 0:gw], func=AF.Gelu,
                            bias=b1_sb[:, hc:hc + 1], scale=1.0)
                    for tr in range(gw // 128):
                        t = (g0 + tr * 128) // 128
                        mp = pstile()
                        nc.tensor.matmul(mp, ones_row[0:1, :], b2_sb,
                                         start=True, stop=False,
                                         skip_group_check=True)
                        for c in range(16):
                            nc.tensor.matmul(
                                mp, hT[:, c, tr * 128:(tr + 1) * 128],
                                w2_sb[:, c, :],
                                start=False, stop=(c == 15),
                                skip_group_check=True)
                        nc.vector.tensor_add(tok[:, t, :], tok[:, t, :], mp)

            # ================= head =================
            hw_sb = lay.tile([128, 4, 4], MMDT, tag='hwsb')
            nc.sync.dma_start(out=hw_sb,
                              in_=hw_d[:, :].rearrange('(c p) n -> p c n', p=128))
            hb_sb = lay.tile([1, 4], MMDT, tag='hbsb')
            nc.sync.dma_start(out=hb_sb, in_=hb_d[None, :])

            cls_sb = lay.tile([NIMG, D], F32, tag='cls')
            for img in range(NIMG):
                r = IMGOFF[img]
                nc.sync.dma_start(out=cls_sb[img:img + 1, :],
                                  in_=tok[r % 128:r % 128 + 1, r // 128, :])
            clsn = lay.tile([NIMG, D], F32, tag='clsn')
            layer_norm_apply(cls_sb[0:NIMG, :], clsn[0:NIMG, :], n_rows=NIMG)
            clsT = lay.tile([128, 4, NIMG], MMDT, tag='clsT')
            for c in range(4):
                tp = pstile()
                nc.tensor.transpose(tp[0:128, 0:NIMG],
                                    clsn[0:NIMG, c * 128:(c + 1) * 128],
                                    ident[0:NIMG, 0:NIMG])
                nc.vector.tensor_copy(out=clsT[:, c, :], in_=tp[0:128, 0:NIMG])
            op = pstile()
            nc.tensor.matmul(op[0:NIMG, 0:4], ones_row[0:1, 0:NIMG], hb_sb,
                             start=True, stop=False, skip_group_check=True)
            for c in range(4):
                nc.tensor.matmul(op[0:NIMG, 0:4], clsT[:, c, :],
                                 hw_sb[:, c, :],
                                 start=False, stop=(c == 3),
                                 skip_group_check=True)
            osb = lay.tile([NIMG, 4], F32, tag='osb')
            nc.vector.tensor_copy(out=osb[0:NIMG, :], in_=op[0:NIMG, 0:4])
            nc.sync.dma_start(out=out_d[:, :], in_=osb[0:NIMG, :])

    return nc


# ============================================================================
# entry point
# ============================================================================
def kernel(**inputs) -> np.ndarray:
    _install_fixups()
    from concourse.bass_utils import run_bass_kernel_spmd

    if 'nc' not in _PROGRAM_CACHE:
        _PROGRAM_CACHE['nc'] = _build_program()
    nc = _PROGRAM_CACHE['nc']

    in_maps = _host_prep(inputs)
    res = run_bass_kernel_spmd(nc, in_maps, core_ids=list(range(NCORES)))
    out = np.concatenate([np.asarray(res.results[i]['out'])
                          for i in range(NCORES)], 0)
    return out[:, :NCLS].astype(np.float32)



# revision 40
# speedup vs baseline: 1.2625x; 1.2625x over previous
"""Lensiformer forward pass on 8 Trainium2 NeuronCores.

Strategy: data-parallel over batch (32 images -> 4 per core, params
replicated, no collectives). Per core, a single fused Bass/Tile program
runs the whole network.

v2 (engine-balance rework of the f32r baseline):
  - bf16 matmul operands everywhere in the transformer (residual stream,
    LN statistics and PSUM accumulation stay fp32)
  - attention masks (self-mask diagonal + tail/pad) are applied by
    accumulating tiny matmuls into the score PSUM group instead of DVE
    tensor-adds
  - score chunks 0/1 share one 2-bank PSUM tile -> single strided exp;
    the 2-row tail chunk's scores live in spare partitions of the pv
    bank; softmax denominator via a 65th all-ones V column
  - 1/denom applied by a DVE multiply that moves pv PSUM -> ot SBUF
    (no separate copy), V bias folded into projb on the host
  - Q^T/K^T computed once for all 4 images (free dim 512)
  - LN rstd = exp(-0.5*ln(var+eps)) so LN + attention exp + copies all
    live in one ACT table; gelu is the only table switch (2/layer)
  - next layer's LN1 stats interleaved into MLP2 emission; its ACT ops
    grouped after the gelus to avoid table thrash
  - PSUM: 4 x 1-bank rotating tiles + 2 x 2-bank score tiles

Self-contained: includes the walrus sync-wait-limit workaround and the
axon NTFF profiling shim.
"""
import contextlib
import ctypes
import sys
import types

import numpy as np
import ml_dtypes

import concourse.bass as bass
import concourse.mybir as mybir
import concourse.tile as tile
from concourse.masks import make_identity
from concourse.vector_clock import ScopedClock

F32 = mybir.dt.float32
F32R = mybir.dt.float32r
BF16 = mybir.dt.bfloat16
AF = mybir.ActivationFunctionType
ALU = mybir.AluOpType

# ---------------- model geometry (hardcoded from the problem spec) ----------
B, IMG, PATCH = 32, 128, 8
D, H, L, MLP, NCLS = 512, 8, 8, 2048, 3
GRID = IMG // PATCH            # 16
P = GRID * GRID                # 256 patches / image
N = P + 1                      # 257 tokens / image
HD = D // H                    # 64
KC = 320                       # im2col contraction: 5 shifts * 8 * 8
NCORES = 8
NIMG = B // NCORES             # 4 images / core
TP = NIMG * P                  # 1024 patch tokens / core
NT = NIMG * N                  # 1028 transformer tokens / core
NTILE = 9                      # token tiles of 128
TT = NTILE * 128               # 1152 padded tokens
IMGOFF = [i * N for i in range(NIMG)]
NEG = -1.0e30
NQ = 258                       # score q width (257 real + 1 pad col)
NQR = 257                      # real q cols
QKW = NT + 1                   # qkt cols (covers the pad col read)
BF_NP = ml_dtypes.bfloat16

_PROGRAM_CACHE = {}

# ============================================================================
# environment fixups
# ============================================================================
_fixups_done = False


def _install_fixups():
    global _fixups_done
    if _fixups_done:
        return
    _fixups_done = True
    MAXW = 1

    def _split_waits(nc, ordered):
        for bb_name, insts in ordered.items():
            new_list = []
            for inst in insts:
                si = getattr(inst, 'sync_info', None)
                eng = getattr(inst, 'engine', None)
                if (si is not None and si.on_wait and len(si.on_wait) > MAXW
                        and eng is not None
                        and type(inst).__name__.startswith('Inst')):
                    waits = list(si.on_wait)
                    inst.sync_info = mybir.SyncInfo(
                        on_wait=waits[:MAXW], on_update=list(si.on_update or []))
                    for i in range(MAXW, len(waits), MAXW):
                        new_list.append(mybir.InstNoOp(
                            name=nc.get_next_instruction_name(),
                            engine=eng, bass_nofuse=True,
                            sync_info=mybir.SyncInfo(
                                on_wait=waits[i:i + MAXW], on_update=[])))
                new_list.append(inst)
            ordered[bb_name] = new_list

    orig_lower = tile.TileContext._lower_ordered_insts

    def patched_lower(self, ordered):
        _split_waits(self.nc, ordered)
        return orig_lower(self, ordered)

    tile.TileContext._lower_ordered_insts = patched_lower

    def patched_drain_and_barrier(self, tick_clock, wait_clock):
        drain_inst = self.nc.sync.drain()
        wait_clock.add_sem_waits(
            drain_inst.ins, ScopedClock({None: tick_clock.global_clock}))
        si = drain_inst.ins.sync_info
        if si and si.on_wait and len(si.on_wait) > MAXW:
            waits = list(si.on_wait)
            drain_inst.ins.sync_info = mybir.SyncInfo(
                on_wait=waits[:MAXW], on_update=list(si.on_update or []))
            for i in range(MAXW, len(waits), MAXW):
                extra = self.nc.sync.drain()
                extra.ins.sync_info = mybir.SyncInfo(
                    on_wait=waits[i:i + MAXW], on_update=[])
        self.nc.all_engine_barrier()
        assert self.sems is not None
        popped = self.nc._tile_sem_poison_stack.pop()
        assert popped is self._sem_poison
        self.nc.clear_and_free_semaphores(list(self.sems.allocated().values()))
        self.nc.all_engine_barrier()

    tile.TileContext._drain_and_barrier = patched_drain_and_barrier

    if 'antenv.axon_hooks' not in sys.modules:
        holder = {'h': None}
        mod = types.ModuleType('antenv.axon_hooks')
        mod.set_axon_ntff_profile_hook = lambda h: holder.__setitem__('h', h)
        mod.get_axon_ntff_profile_hook = lambda: holder['h']
        sys.modules['antenv.axon_hooks'] = mod
        try:
            lib = ctypes.CDLL('/opt/axon/libaxon_pjrt.so')
            if hasattr(lib, 'axon_start_nrt_profile'):
                lib.axon_start_nrt_profile.argtypes = [
                    ctypes.POINTER(ctypes.c_int64), ctypes.c_size_t]
                lib.axon_start_nrt_profile.restype = ctypes.c_int64
                lib.axon_stop_nrt_profile.argtypes = [ctypes.c_char_p]
                lib.axon_stop_nrt_profile.restype = ctypes.c_int64

                @contextlib.contextmanager
                def _hook(output_dir, device_ids):
                    import jax
                    jax.devices()
                    if device_ids:
                        ids = (ctypes.c_int64 * len(device_ids))(*device_ids)
                        rc = lib.axon_start_nrt_profile(ids, len(device_ids))
                    else:
                        rc = lib.axon_start_nrt_profile(None, 0)
                    if rc != 0:
                        raise RuntimeError(f'axon_start_nrt_profile rc={rc}')
                    try:
                        yield
                    finally:
                        lib.axon_stop_nrt_profile(output_dir.encode())

                mod.set_axon_ntff_profile_hook(_hook)
        except OSError:
            pass


# ============================================================================
# host-side input marshaling (pure data movement + tiny param folds)
# ============================================================================
def _im2col(image):
    """(Bc,1,IMG,IMG) -> (Bc, P, 320), col order [shift, py, px]."""
    shifts = [(0, 0), (1, 1), (-1, 1), (1, -1), (-1, -1)]
    x = image[:, 0]
    cols = []
    for (sy, sx) in shifts:
        xs = np.roll(x, (sy, sx), (1, 2))
        pt = xs.reshape(-1, GRID, PATCH, GRID, PATCH).transpose(0, 1, 3, 2, 4)
        cols.append(pt.reshape(-1, P, PATCH * PATCH))
    return np.concatenate(cols, -1)


def _bf(a):
    return np.ascontiguousarray(np.asarray(a, np.float32).astype(BF_NP))


def _host_prep(inputs):
    f = lambda k: np.ascontiguousarray(np.asarray(inputs[k], np.float32))
    image = f('image')

    # conv weights -> matmul form, both tokenizers side by side
    wconv = np.concatenate(
        [f('ssw').reshape(D, KC).T, f('sow').reshape(D, KC).T], 1)  # (320,1024)
    bconv = np.concatenate([f('ssb'), f('sob')])                    # (1024,)
    gbeta = np.stack([np.concatenate([f('ssg'), f('sog')]),
                      np.concatenate([f('ssbeta'), f('sobeta')])])  # (2,1024)

    # fold LN gains/biases into the following matmuls (exact rewrite)
    ln1g, ln1b = f('ln1g'), f('ln1b')
    ln2g, ln2b = f('ln2g'), f('ln2b')
    qkvw, qkvb = f('qkvw'), f('qkvb')
    projw, projb = f('projw'), f('projb')
    w1, b1 = f('w1'), f('b1')
    qkvw_eff = ln1g[:, :, None] * qkvw
    qkvb_eff = qkvb + np.einsum('ld,ldn->ln', ln1b, qkvw)
    w1_eff = ln2g[:, :, None] * w1
    b1_eff = b1 + np.einsum('ld,ldn->ln', ln2b, w1)
    hw_eff = f('ng')[:, None] * f('hw')
    # V bias contributes exactly bv @ projw to the proj output (softmax
    # weights sum to 1 after the 1/denom divide) -> fold into projb
    bv = qkvb_eff[:, 2 * D:3 * D]                      # (L, 512)
    projb_eff = projb + np.einsum('ld,ldn->ln', bv, projw)

    # pos/cls in padded transformer layout
    pos = f('pos_embed')[0]          # (257, 512)
    cls_eff = f('cls_token')[0, 0] + pos[0]
    pospad = np.zeros((TT, D), np.float32)
    for i in range(NIMG):
        pospad[IMGOFF[i]] = cls_eff
        pospad[IMGOFF[i] + 1: IMGOFF[i] + N] = pos[1:]

    X = _im2col(image)               # (B, P, 320)

    common = dict(
        wconv=_bf(wconv), bconv=_bf(bconv), gbeta=_bf(gbeta),
        fw=_bf(f('fw')), fb=_bf(f('fb')), pospad=pospad,
        qkvw=_bf(qkvw_eff),
        qkvbqk=np.ascontiguousarray(qkvb_eff[:, 0:2 * D]),
        projw=_bf(projw), projb=_bf(projb_eff), temp=f('temp'),
        w1=_bf(w1_eff), b1=np.ascontiguousarray(b1_eff),
        w2=_bf(f('w2')), b2=_bf(f('b2')),
        hw=np.ascontiguousarray(
            np.concatenate([hw_eff, np.zeros((D, 1), np.float32)], 1)),
        hb=np.ascontiguousarray(
            np.concatenate([f('hb') + f('nb') @ f('hw'),
                            np.zeros(1, np.float32)])),
    )
    in_maps = []
    for c in range(NCORES):
        # token-layout im2col: col = transformer token index, cls cols zero
        xt = np.zeros((KC, NT), np.float32)
        for i in range(NIMG):
            xt[:, IMGOFF[i] + 1:IMGOFF[i] + N] = X[c * NIMG + i].T
        m = dict(common)
        m['xt'] = _bf(xt)
        in_maps.append(m)
    return in_maps


# ============================================================================
# device program
# ============================================================================
def _tile_segments(t):
    """Real-token segments of token-tile t: (row_in_tile, n, img, pos0)."""
    segs = []
    r0 = 128 * t
    for img in range(NIMG):
        lo = max(r0, IMGOFF[img])
        hi = min(r0 + 128, IMGOFF[img] + N, NT)
        if lo < hi:
            segs.append((lo - r0, hi - lo, img, lo - IMGOFF[img]))
    return segs


def _tile_rows(t):
    """Number of real token rows in token-tile t."""
    return min(128, max(0, NT - 128 * t))


def _build_program():
    nc = bass.Bass()

    din = lambda nm, sh, dt_=F32: nc.dram_tensor(nm, sh, dt_, kind='ExternalInput')
    xt_d = din('xt', [KC, NT], BF16)
    wc_d = din('wconv', [KC, 2 * D], BF16)
    bc_d = din('bconv', [2 * D], BF16)
    gb_d = din('gbeta', [2, 2 * D], BF16)
    fw_d = din('fw', [2 * D, D], BF16)
    fb_d = din('fb', [D], BF16)
    pos_d = din('pospad', [TT, D])
    qkvw_d = din('qkvw', [L, D, 3 * D], BF16)
    qkvbqk_d = din('qkvbqk', [L, 2 * D])
    projw_d = din('projw', [L, D, D], BF16)
    projb_d = din('projb', [L, D], BF16)
    temp_d = din('temp', [L, H])
    w1_d = din('w1', [L, D, MLP], BF16)
    b1_d = din('b1', [L, MLP])
    w2_d = din('w2', [L, MLP, D], BF16)
    b2_d = din('b2', [L, D], BF16)
    hw_d = din('hw', [D, 4])
    hb_d = din('hb', [4])
    out_d = nc.dram_tensor('out', [NIMG, 4], F32, kind='ExternalOutput')

    with tile.TileContext(nc) as tc, \
            nc.allow_low_precision(reason='bf16 matmul operands'):
        with contextlib.ExitStack() as ctx:
            sb = ctx.enter_context(tc.tile_pool(name='sb', bufs=1))
            ps = ctx.enter_context(tc.tile_pool(name='ps', bufs=4, space='PSUM'))
            ps2 = ctx.enter_context(tc.tile_pool(name='ps2', bufs=2, space='PSUM'))

            _psn = [0]

            def pstile(dt_=F32):
                _psn[0] += 1
                return ps.tile([128, 512], dt_, tag='ps', bufs=4,
                               name=f'ps{_psn[0]}')

            # ---------------- constants ----------------
            ident = sb.tile([128, 128], F32, tag='ident')
            make_identity(nc, ident)
            ident_bf = sb.tile([128, 128], BF16, tag='identbf')
            nc.vector.tensor_copy(out=ident_bf, in_=ident)
            negid_bf = sb.tile([128, 128], BF16, tag='negidbf')
            nc.scalar.mul(out=negid_bf, in_=ident_bf, mul=NEG)
            c2negr = sb.tile([2, NQ], BF16, tag='c2negr')
            nc.vector.memset(c2negr, NEG)
            nc.vector.memset(c2negr[0:1, :], 0.0)
            nc.vector.memset(c2negr[0:1, 256:257], NEG)
            ones_row = sb.tile([1, 128], BF16, tag='ones_row')
            nc.vector.memset(ones_row, 1.0)
            eps = sb.tile([128, 1], F32, tag='eps')
            nc.vector.memset(eps, 1e-5)

            # ---------------- persistent activations ----------------
            tok = sb.tile([128, NTILE, D], F32, tag='tok')       # residual

            lay = None  # transformer pool; opened after the embed pool closes

            # ---- LN helpers (split so ACT parts can be emission-grouped) ---
            def ln_stats(t):
                stats = lay.tile([128, 6], F32, tag='lnstat', bufs=6)
                mv = lay.tile([128, 2], F32, tag='lnmv', bufs=6)
                nc.vector.bn_stats(out=stats, in_=tok[:, t, :])
                nc.vector.bn_aggr(out=mv, in_=stats)
                return mv

            def ln_rstd(mv):
                """Ln/Exp rstd chain (the ACT-table-sensitive part)."""
                lnv = lay.tile([128, 1], F32, tag='lnv', bufs=10)
                nc.scalar.activation(out=lnv, in_=mv[:, 1:2], func=AF.Ln,
                                     bias=eps, scale=1.0)
                rstd = lay.tile([128, 1], F32, tag='lnrstd', bufs=10)
                nc.scalar.activation(out=rstd, in_=lnv, func=AF.Exp,
                                     scale=-0.5)
                nmr = lay.tile([128, 1], F32, tag='lnnmr', bufs=10)
                nc.vector.scalar_tensor_tensor(
                    out=nmr, in0=mv[:, 0:1], scalar=-1.0,
                    in1=rstd, op0=ALU.mult, op1=ALU.mult)
                return rstd, nmr

            def ln_apply_tr(t, rstd, nmr, xT_dst):
                """Identity apply + transposes + copy (table-neutral)."""
                xn = lay.tile([128, D], BF16, tag='xn', bufs=3)
                nc.scalar.activation(out=xn, in_=tok[:, t, :],
                                     func=AF.Identity, scale=rstd, bias=nmr)
                tp = pstile(BF16)
                for c in range(4):
                    nc.tensor.transpose(tp[:, c * 128:(c + 1) * 128],
                                        xn[:, c * 128:(c + 1) * 128], ident_bf)
                nc.vector.tensor_copy(
                    out=xT_dst[:, :, t * 128:(t + 1) * 128], in_=tp)

            def ln_stats_rstd(t):
                """stats + rstd chain for one tile, emitted at the point the
                tile's residual add lands (ACT table there is exp-compatible
                in the attn/proj phase; in the MLP G-loop it costs the one
                switch back from the gelu table that the next phase needs
                anyway)."""
                return ln_rstd(ln_stats(t))

            # tile groups matching the 128/384/512/tail column groups that
            # B_qk / MLP1 consume; applies interleave with those groups
            _LN_GROUPS = ((0,), (1, 2, 3), (4, 5, 6, 7), (8,))

            def ln_sweep(rn, xT_dst, group_emitters=None):
                for gi, tiles in enumerate(_LN_GROUPS):
                    for t in tiles:
                        ln_apply_tr(t, rn[t][0], rn[t][1], xT_dst)
                    if group_emitters is not None:
                        group_emitters[gi]()

            # ================= patch embed =================
            # the im2col input is already in transformer token layout (cls
            # cols zero), so conv/gate/fuse run per token tile and tokens
            # land in `tok` without a DRAM reshuffle
            with tc.tile_pool(name='emb', bufs=1) as emb:
                xt_sb = []
                for kc, k0, kn in ((0, 0, 128), (1, 128, 128), (2, 256, 64)):
                    t_ = emb.tile([kn, NT], BF16, tag=f'xt{kc}')
                    nc.sync.dma_start(out=t_, in_=xt_d[k0:k0 + kn, :])
                    xt_sb.append(t_)
                wc_sb = []
                for kc, k0, kn in ((0, 0, 128), (1, 128, 128), (2, 256, 64)):
                    t_ = emb.tile([kn, 2 * D], BF16, tag=f'wc{kc}')
                    nc.sync.dma_start(out=t_, in_=wc_d[k0:k0 + kn, :])
                    wc_sb.append(t_)
                bc_sb = emb.tile([1, 2 * D], BF16, tag='bc')
                nc.sync.dma_start(out=bc_sb, in_=bc_d[None, :])
                gb_g = emb.tile([128, 2 * D], BF16, tag='gbg')
                nc.sync.dma_start(
                    out=gb_g, in_=gb_d[0][None, :].to_broadcast([128, 2 * D]))
                gb_b = emb.tile([128, 2 * D], BF16, tag='gbb')
                nc.sync.dma_start(
                    out=gb_b, in_=gb_d[1][None, :].to_broadcast([128, 2 * D]))
                fw_sb = emb.tile([128, 8, D], BF16, tag='fwsb')
                nc.sync.dma_start(
                    out=fw_sb, in_=fw_d[:, :].rearrange('(c p) n -> p c n', p=128))
                fb_sb = emb.tile([1, D], BF16, tag='fbsb')
                nc.sync.dma_start(out=fb_sb, in_=fb_d[None, :])

                # cls token rows per token tile: (tile, row)
                cls_rows = {IMGOFF[i] // 128: IMGOFF[i] % 128
                            for i in range(NIMG)}
                nc.vector.memset(tok[:, NTILE - 1, :], 0.0)
                combs, ggs = [], []
                # sweep 1: conv + LN + gains + transposes + gate matmul
                # (ACT stays in the ln/exp/identity table)
                for t in range(NTILE):
                    rows = _tile_rows(t)
                    c0 = t * 128
                    combraw = emb.tile([128, 2 * D], F32, tag='combraw', bufs=3)
                    for nh in range(2):
                        cps = pstile()
                        nc.tensor.matmul(cps[0:rows, :], ones_row[0:1, 0:rows],
                                         bc_sb[0:1, nh * D:(nh + 1) * D],
                                         start=True, stop=False,
                                         skip_group_check=True)
                        for kc in range(3):
                            nc.tensor.matmul(
                                cps[0:rows, :], xt_sb[kc][:, c0:c0 + rows],
                                wc_sb[kc][:, nh * D:(nh + 1) * D],
                                start=False, stop=(kc == 2),
                                skip_group_check=True)
                        nc.vector.tensor_copy(
                            out=combraw[0:rows, nh * D:(nh + 1) * D],
                            in_=cps[0:rows, :])

                    comb = emb.tile([128, 2 * D], BF16, tag='comb', bufs=9)
                    for nh in range(2):
                        sl = slice(nh * D, (nh + 1) * D)
                        stats = emb.tile([128, 6], F32, tag='estat', bufs=4)
                        mv = emb.tile([128, 2], F32, tag='emv', bufs=4)
                        nc.vector.bn_stats(out=stats[0:rows],
                                           in_=combraw[0:rows, sl])
                        nc.vector.bn_aggr(out=mv[0:rows], in_=stats[0:rows])
                        lnv = emb.tile([128, 1], F32, tag='elnv', bufs=4)
                        nc.scalar.activation(out=lnv[0:rows],
                                             in_=mv[0:rows, 1:2],
                                             func=AF.Ln, bias=eps[0:rows],
                                             scale=1.0)
                        rstd = emb.tile([128, 1], F32, tag='erstd', bufs=4)
                        nc.scalar.activation(out=rstd[0:rows], in_=lnv[0:rows],
                                             func=AF.Exp, scale=-0.5)
                        nmr = emb.tile([128, 1], F32, tag='enmr', bufs=4)
                        nc.vector.scalar_tensor_tensor(
                            out=nmr[0:rows], in0=mv[0:rows, 0:1], scalar=-1.0,
                            in1=rstd[0:rows], op0=ALU.mult, op1=ALU.mult)
                        nc.scalar.activation(out=comb[0:rows, sl],
                                             in_=combraw[0:rows, sl],
                                             func=AF.Identity,
                                             scale=rstd[0:rows],
                                             bias=nmr[0:rows])
                    nc.vector.tensor_mul(comb[0:rows], comb[0:rows], gb_g[0:rows])
                    nc.vector.tensor_add(comb[0:rows], comb[0:rows], gb_b[0:rows])

                    combT = emb.tile([128, 8, 128], BF16, tag='combT', bufs=3)
                    for half in range(2):
                        tp = ps2.tile([128, 2, 512], BF16, tag='stp2', bufs=2,
                                      name=f'etp{t}_{half}')
                        for c in range(4):
                            cc = half * 4 + c
                            nc.tensor.transpose(
                                tp[:, 0, c * rows:(c + 1) * rows],
                                comb[0:rows, cc * 128:(cc + 1) * 128],
                                ident_bf[0:rows, 0:rows])
                        nc.vector.tensor_copy(
                            out=combT[:, half * 4:(half + 1) * 4, 0:rows],
                            in_=tp[:, 0, 0:4 * rows].rearrange(
                                'p (c n) -> p c n', c=4))

                    gps = pstile()
                    nc.tensor.matmul(gps[0:rows, :], ones_row[0:1, 0:rows],
                                     fb_sb, start=True, stop=False,
                                     skip_group_check=True)
                    for c in range(8):
                        nc.tensor.matmul(gps[0:rows, :], combT[:, c, 0:rows],
                                         fw_sb[:, c, :],
                                         start=False, stop=(c == 7),
                                         skip_group_check=True)
                    gg = emb.tile([128, D], F32, tag='gg', bufs=9)
                    nc.scalar.copy(out=gg[0:rows], in_=gps[0:rows, :])
                    combs.append(comb)
                    ggs.append(gg)

                # sweep 2: sigmoids (single ACT table switch), then fuse into
                # tok (cls rows zeroed; +pos gives cls_eff there)
                for t in range(NTILE):
                    rows = _tile_rows(t)
                    gt = emb.tile([128, D], F32, tag='gt', bufs=2)
                    nc.scalar.activation(out=gt[0:rows], in_=ggs[t][0:rows],
                                         func=AF.Sigmoid)
                    diff = emb.tile([128, D], F32, tag='diff', bufs=2)
                    nc.vector.tensor_sub(diff[0:rows], combs[t][0:rows, 0:D],
                                         combs[t][0:rows, D:2 * D])
                    nc.vector.tensor_mul(diff[0:rows], diff[0:rows],
                                         gt[0:rows])
                    nc.vector.tensor_add(diff[0:rows], diff[0:rows],
                                         combs[t][0:rows, D:2 * D])
                    postile = emb.tile([128, D], F32, tag='pos', bufs=3)
                    nc.gpsimd.dma_start(out=postile,
                                        in_=pos_d[t * 128:(t + 1) * 128, :])
                    nc.vector.tensor_add(tok[0:rows, t, :], diff[0:rows],
                                         postile[0:rows])
                    if t in cls_rows:
                        # cls rows carry conv-of-zeros junk; overwrite with
                        # cls_eff straight from pospad (DMA may address any
                        # partition, unlike the compute engines)
                        r = cls_rows[t]
                        nc.sync.dma_start(
                            out=tok[r:r + 1, t, :],
                            in_=pos_d[t * 128 + r:t * 128 + r + 1, :])

            # ================= transformer layers =================
            lay = ctx.enter_context(tc.tile_pool(name='lay', bufs=1))

            # after C(img), ot cols for these token tiles are complete
            _D_TILES = {0: (0, 1), 1: (2, 3), 2: (4, 5), 3: (6, 7, 8)}

            _BQK_GROUPS = ((0, 128), (128, 384), (512, 512),
                           (1024, QKW - 1024))

            def bqk_group(qkvw_sb, qkvb_sb, qkt, xT, gi):
                g0, gw = _BQK_GROUPS[gi]

                def emit():
                    for fc in range(8):
                        qp = pstile()
                        for c in range(4):
                            nc.tensor.matmul(
                                qp[:, 0:gw],
                                qkvw_sb[:, c, fc * 128:(fc + 1) * 128],
                                xT[:, c, g0:g0 + gw],
                                start=(c == 0), stop=(c == 3))
                        nc.scalar.activation(
                            out=qkt[:, fc, g0:g0 + gw], in_=qp[:, 0:gw],
                            func=AF.Identity, bias=qkvb_sb[:, fc:fc + 1],
                            scale=1.0)
                return emit

            def attn_proj_phase(l, xT, qkt, qkvw_sb):
                """B_v + C + proj(+residual) for layer l (B_qk for this layer
                ran fused into the previous MLP2 loop). Returns LN2 rstd
                chains per tile."""
                temp_sb = lay.tile([128, H], F32, tag='temp', bufs=2)
                nc.sync.dma_start(out=temp_sb,
                                  in_=temp_d[l][None, :].to_broadcast([128, H]))
                projw_sb = lay.tile([128, 4, D], BF16, tag='wt', bufs=3)
                nc.sync.dma_start(
                    out=projw_sb,
                    in_=projw_d[l].rearrange('(c p) n -> p c n', p=128))
                projb_sb = lay.tile([1, D], BF16, tag='projb', bufs=2)
                nc.sync.dma_start(out=projb_sb, in_=projb_d[l][None, :])

                # ---- B_v + C + proj, interleaved per image ----
                vimg = lay.tile([128, NIMG, 2, H * 65], BF16, tag='vimg',
                                bufs=1)
                nc.gpsimd.memset(
                    vimg.rearrange('p i c (h e) -> p i c h e', e=65)
                    [:, :, :, :, 64:65], 1.0)
                vl = lay.tile([2, NIMG, H * 65], BF16, tag='vlast', bufs=1)
                nc.gpsimd.memset(vl, 0.0)
                nc.gpsimd.memset(
                    vl.rearrange('p i (h e) -> p i h e', e=65)
                    [:, :, :, 64:65], 1.0)
                ot = lay.tile([128, 4, NT], BF16, tag='ot', bufs=2)
                rn2 = {}
                for img in range(NIMG):
                    io = IMGOFF[img]
                    # B_v for this image
                    for c2 in range(2):
                        vp = pstile()
                        for c in range(4):
                            nc.tensor.matmul(
                                vp, xT[:, c, io + c2 * 128:io + (c2 + 1) * 128],
                                qkvw_sb[:, c, 2 * D:3 * D],
                                start=(c == 0), stop=(c == 3))
                        nc.scalar.copy(
                            out=vimg.rearrange(
                                'p i c (h e) -> p i c h e', e=65)
                            [:, img, c2, :, 0:64],
                            in_=vp)
                    vp2 = pstile()
                    for c in range(4):
                        nc.tensor.matmul(vp2[0:1, :],
                                         xT[:, c, io + P:io + P + 1],
                                         qkvw_sb[:, c, 2 * D:3 * D],
                                         start=(c == 0), stop=(c == 3))
                    nc.scalar.copy(
                        out=vl.rearrange('p i (h e) -> p i h e', e=65)
                        [0:1, img, :, 0:64],
                        in_=vp2[0:1, :])
                    # C: attention for this image (transposed softmax)
                    for h in range(H):
                        hr = (h % 2) * 64
                        qfc = h // 2
                        kfc = 4 + h // 2
                        s2 = ps2.tile([128, 2, 512], F32, tag='stp2', bufs=2)
                        pv = pstile()
                        for c in range(2):
                            nc.tensor.matmul(
                                s2[:, c, 0:NQ],
                                qkt[hr:hr + 64, kfc, io + c * 128:io + (c + 1) * 128],
                                qkt[hr:hr + 64, qfc, io:io + NQ],
                                start=True, stop=False, skip_group_check=True)
                            nc.tensor.matmul(
                                s2[:, c, c * 128:(c + 1) * 128],
                                ident_bf, negid_bf,
                                start=False, stop=True, skip_group_check=True)
                        # tail-chunk scores borrow pv rows 32:34; the pv
                        # group overwrites them after their exp is consumed
                        nc.tensor.matmul(
                            pv[32:34, 0:NQ],
                            qkt[hr:hr + 64, kfc, io + 256:io + 258],
                            qkt[hr:hr + 64, qfc, io:io + NQ],
                            start=True, stop=False, skip_group_check=True)
                        nc.tensor.matmul(
                            pv[32:34, 0:NQ], ident_bf[0:2, 0:2], c2negr,
                            start=False, stop=True, skip_group_check=True)
                        pt = lay.tile([128, 3, NQ], BF16, tag='pt', bufs=3)
                        nc.scalar.activation(
                            out=pt[:, 0:2, :], in_=s2[:, :, 0:NQ],
                            func=AF.Exp, scale=temp_sb[:, h:h + 1])
                        nc.scalar.activation(
                            out=pt[0:2, 2, :], in_=pv[32:34, 0:NQ],
                            func=AF.Exp, scale=temp_sb[32:34, h:h + 1])
                        for c in range(3):
                            cm = (128, 128, 2)[c]
                            lhs = (vimg[0:cm, img, c, h * 65:h * 65 + 65]
                                   if c < 2
                                   else vl[0:2, img, h * 65:h * 65 + 65])
                            nc.tensor.matmul(
                                pv[0:65, 0:NQ], lhs, pt[0:cm, c, :],
                                start=(c == 0), stop=(c == 2),
                                skip_group_check=True)
                        rr = lay.tile([1, NQ], BF16, tag='rr', bufs=4)
                        nc.vector.reciprocal(out=rr[0:1, 0:NQR],
                                             in_=pv[64:65, 0:NQR])
                        # broadcast 1/denom into pv rows 64:128 (the denom
                        # row is dead once the reciprocal has read it), then
                        # stage it in SBUF (DVE may read only one PSUM input)
                        nc.tensor.matmul(pv[64:128, 0:NQR],
                                         ones_row[0:1, 0:64], rr[0:1, 0:NQR],
                                         start=True, stop=True,
                                         skip_group_check=True)
                        rts = lay.tile([64, NQ], BF16, tag='rts', bufs=4)
                        nc.scalar.copy(out=rts[0:64, 0:NQR],
                                       in_=pv[64:128, 0:NQR])
                        nc.vector.tensor_mul(ot[hr:hr + 64, qfc, io:io + NQR],
                                             pv[0:64, 0:NQR],
                                             rts[0:64, 0:NQR])
                    # proj + residual for token tiles completed by this image
                    for t in _D_TILES[img]:
                        rows = _tile_rows(t)
                        pp = pstile()
                        nc.tensor.matmul(pp[0:rows, :], ones_row[0:1, 0:rows],
                                         projb_sb, start=True, stop=False,
                                         skip_group_check=True)
                        for c in range(4):
                            nc.tensor.matmul(
                                pp[0:rows, :],
                                ot[:, c, t * 128:t * 128 + rows],
                                projw_sb[:, c, :],
                                start=False, stop=(c == 3),
                                skip_group_check=True)
                        nc.vector.tensor_add(tok[0:rows, t, :],
                                             tok[0:rows, t, :], pp[0:rows, :])
                        rn2[t] = ln_stats_rstd(t)
                return rn2

            def mlp_phase(l, xT2, ln2_rn, want_next):
                """LN2-apply + MLP(+residual) for layer l, with next layer's
                LN1 applies + B_qk fused into the MLP2 loop per tile group.
                Returns (xT_next, qkt_next, qkvw_next, ln1_rn_next)."""
                w1_sb = lay.tile([128, 4, MLP], BF16, tag='wt', bufs=3)
                nc.sync.dma_start(
                    out=w1_sb, in_=w1_d[l].rearrange('(c p) n -> p c n', p=128))
                b1_sb = lay.tile([128, 16], F32, tag='b1', bufs=2)
                nc.sync.dma_start(
                    out=b1_sb, in_=b1_d[l].rearrange('(c p) -> p c', p=128))
                w2_sb = lay.tile([128, 16, D], BF16, tag='wt', bufs=3)
                nc.sync.dma_start(
                    out=w2_sb, in_=w2_d[l].rearrange('(c p) n -> p c n', p=128))
                b2_sb = lay.tile([1, D], BF16, tag='b2', bufs=2)
                nc.sync.dma_start(out=b2_sb, in_=b2_d[l][None, :])
                if want_next:
                    qkvw_n = lay.tile([128, 4, 3 * D], BF16, tag='wt', bufs=3)
                    nc.sync.dma_start(
                        out=qkvw_n,
                        in_=qkvw_d[l + 1].rearrange('(c p) n -> p c n', p=128))
                    qkvb_n = lay.tile([128, 8], F32, tag='qkvb', bufs=2)
                    nc.sync.dma_start(
                        out=qkvb_n,
                        in_=qkvbqk_d[l + 1].rearrange('(c p) -> p c', p=128))
                    xT_n = lay.tile([128, 4, TT], BF16, tag='xT', bufs=2)
                    qkt_n = lay.tile([128, 8, QKW], BF16, tag='qkt', bufs=2)
                else:
                    qkvw_n = qkvb_n = xT_n = qkt_n = None

                groups = ((0, 128), (128, 384), (512, 512), (1024, NT - 1024))
                hTs = {}

                def f_group(gi, g0, gw):
                    def emit():
                        hT = lay.tile([128, 16, gw], BF16,
                                      tag=('hT' if gw >= 384 else 'hTs'),
                                      bufs=2, name=f'hT{l}_{gi}')
                        hTs[gi] = hT
                        for hc in range(16):
                            hp = pstile()
                            for c in range(4):
                                nc.tensor.matmul(
                                    hp[:, 0:gw],
                                    w1_sb[:, c, hc * 128:(hc + 1) * 128],
                                    xT2[:, c, g0:g0 + gw],
                                    start=(c == 0), stop=(c == 3))
                            nc.scalar.activation(
                                out=hT[:, hc, :], in_=hp[:, 0:gw],
                                func=AF.Gelu, bias=b1_sb[:, hc:hc + 1],
                                scale=1.0)
                    return emit

                ln_sweep(ln2_rn, xT2,
                         [f_group(gi, g0, gw)
                          for gi, (g0, gw) in enumerate(groups)])

                rn_next = {}
                for gi, (g0, gw) in enumerate(groups):
                    hT = hTs[gi]
                    ntr = (gw + 127) // 128
                    for tr in range(ntr):
                        t = (g0 + tr * 128) // 128
                        rows = min(128, gw - tr * 128)
                        mp = pstile()
                        nc.tensor.matmul(mp[0:rows, :], ones_row[0:1, 0:rows],
                                         b2_sb, start=True, stop=False,
                                         skip_group_check=True)
                        for c in range(16):
                            nc.tensor.matmul(
                                mp[0:rows, :],
                                hT[:, c, tr * 128:tr * 128 + rows],
                                w2_sb[:, c, :],
                                start=False, stop=(c == 15),
                                skip_group_check=True)
                        nc.vector.tensor_add(tok[0:rows, t, :],
                                             tok[0:rows, t, :], mp[0:rows, :])
                        if want_next:
                            rn_next[t] = ln_stats_rstd(t)
                if want_next:
                    ln_sweep(rn_next, xT_n,
                             [bqk_group(qkvw_n, qkvb_n, qkt_n, xT_n, gi)
                              for gi in range(4)])
                return xT_n, qkt_n, qkvw_n, rn_next

            # layer 0 prologue: LN1 + B_qk
            rn1 = {t: ln_stats_rstd(t) for t in range(NTILE)}
            qkvw_sb = lay.tile([128, 4, 3 * D], BF16, tag='wt', bufs=3)
            nc.sync.dma_start(
                out=qkvw_sb,
                in_=qkvw_d[0].rearrange('(c p) n -> p c n', p=128))
            qkvb_sb = lay.tile([128, 8], F32, tag='qkvb', bufs=2)
            nc.sync.dma_start(
                out=qkvb_sb,
                in_=qkvbqk_d[0].rearrange('(c p) -> p c', p=128))
            xT = lay.tile([128, 4, TT], BF16, tag='xT', bufs=2)
            qkt = lay.tile([128, 8, QKW], BF16, tag='qkt', bufs=2)
            ln_sweep(rn1, xT,
                     [bqk_group(qkvw_sb, qkvb_sb, qkt, xT, gi)
                      for gi in range(4)])
            for l in range(L):
                rn2 = attn_proj_phase(l, xT, qkt, qkvw_sb)
                xT2 = lay.tile([128, 4, TT], BF16, tag='xT', bufs=2)
                xT, qkt, qkvw_sb, rn1 = mlp_phase(l, xT2, rn2, l < L - 1)

            # ================= head =================
            hw_sb = lay.tile([128, 4, 4], F32, tag='hwsb')
            nc.sync.dma_start(out=hw_sb,
                              in_=hw_d[:, :].rearrange('(c p) n -> p c n', p=128))
            hb_sb = lay.tile([1, 4], F32, tag='hbsb')
            nc.sync.dma_start(out=hb_sb, in_=hb_d[None, :])

            cls_sb = lay.tile([NIMG, D], F32, tag='cls')
            for img in range(NIMG):
                r = IMGOFF[img]
                nc.sync.dma_start(out=cls_sb[img:img + 1, :],
                                  in_=tok[r % 128:r % 128 + 1, r // 128, :])
            # final LN on the 4 cls tokens
            stats = lay.tile([NIMG, 6], F32, tag='hstat')
            mv = lay.tile([NIMG, 2], F32, tag='hmv')
            nc.vector.bn_stats(out=stats, in_=cls_sb[0:NIMG, :])
            nc.vector.bn_aggr(out=mv, in_=stats)
            lnv = lay.tile([NIMG, 1], F32, tag='hlnv')
            nc.scalar.activation(out=lnv, in_=mv[:, 1:2], func=AF.Ln,
                                 bias=eps[0:NIMG], scale=1.0)
            rstd = lay.tile([NIMG, 1], F32, tag='hrstd')
            nc.scalar.activation(out=rstd, in_=lnv, func=AF.Exp, scale=-0.5)
            nmr = lay.tile([NIMG, 1], F32, tag='hnmr')
            nc.vector.scalar_tensor_tensor(
                out=nmr, in0=mv[:, 0:1], scalar=-1.0,
                in1=rstd, op0=ALU.mult, op1=ALU.mult)
            clsn = lay.tile([NIMG, D], F32, tag='clsn')
            nc.scalar.activation(out=clsn, in_=cls_sb[0:NIMG, :],
                                 func=AF.Identity, scale=rstd, bias=nmr)
            clsT = lay.tile([128, 4, NIMG], F32, tag='clsT')
            for c in range(4):
                tp = pstile()
                nc.tensor.transpose(tp[0:128, 0:NIMG],
                                    clsn[0:NIMG, c * 128:(c + 1) * 128],
                                    ident[0:NIMG, 0:NIMG])
                nc.vector.tensor_copy(out=clsT[:, c, :], in_=tp[0:128, 0:NIMG])
            onesf = lay.tile([1, NIMG], F32, tag='onesf')
            nc.vector.memset(onesf, 1.0)
            op = pstile()
            nc.tensor.matmul(op[0:NIMG, 0:4], onesf[0:1, 0:NIMG], hb_sb,
                             start=True, stop=False, skip_group_check=True)
            for c in range(4):
                nc.tensor.matmul(op[0:NIMG, 0:4], clsT[:, c, :],
                                 hw_sb[:, c, :],
                                 start=False, stop=(c == 3),
                                 skip_group_check=True)
            osb = lay.tile([NIMG, 4], F32, tag='osb')
            nc.vector.tensor_copy(out=osb[0:NIMG, :], in_=op[0:NIMG, 0:4])
            nc.sync.dma_start(out=out_d[:, :], in_=osb[0:NIMG, :])

    return nc


# ============================================================================
# entry point
# ============================================================================
def kernel(**inputs) -> np.ndarray:
    _install_fixups()
    from concourse.bass_utils import run_bass_kernel_spmd

    if 'nc' not in _PROGRAM_CACHE:
        _PROGRAM_CACHE['nc'] = _build_program()
    nc = _PROGRAM_CACHE['nc']

    in_maps = _host_prep(inputs)
    res = run_bass_kernel_spmd(nc, in_maps, core_ids=list(range(NCORES)))
    out = np.concatenate([np.asarray(res.results[i]['out'])
                          for i in range(NCORES)], 0)
    return out[:, :NCLS].astype(np.float32)


# revision 52
# speedup vs baseline: 1.4265x; 1.1300x over previous
"""Lensiformer forward pass on 8 Trainium2 NeuronCores.

Strategy: data-parallel over batch (32 images -> 4 per core, params
replicated, no collectives). Per core, a single fused Bass/Tile program
runs the whole network.

v2 (engine-balance rework of the f32r baseline):
  - bf16 matmul operands everywhere in the transformer (residual stream,
    LN statistics and PSUM accumulation stay fp32)
  - attention masks (self-mask diagonal + tail/pad) are applied by
    accumulating tiny matmuls into the score PSUM group instead of DVE
    tensor-adds
  - score chunks 0/1 share one 2-bank PSUM tile -> single strided exp;
    the 2-row tail chunk's scores live in spare partitions of the pv
    bank; softmax denominator via a 65th all-ones V column
  - 1/denom applied by a DVE multiply that moves pv PSUM -> ot SBUF
    (no separate copy), V bias folded into projb on the host
  - Q^T/K^T computed once for all 4 images (free dim 512)
  - LN rstd = exp(-0.5*ln(var+eps)) so LN + attention exp + copies all
    live in one ACT table; gelu is the only table switch (2/layer)
  - next layer's LN1 stats interleaved into MLP2 emission; its ACT ops
    grouped after the gelus to avoid table thrash
  - PSUM: 4 x 1-bank rotating tiles + 2 x 2-bank score tiles

Self-contained: includes the walrus sync-wait-limit workaround and the
axon NTFF profiling shim.
"""
import contextlib
import ctypes
import sys
import types

import numpy as np
import ml_dtypes

import concourse.bass as bass
import concourse.mybir as mybir
import concourse.tile as tile
from concourse.masks import make_identity
from concourse.vector_clock import ScopedClock

F32 = mybir.dt.float32
F32R = mybir.dt.float32r
BF16 = mybir.dt.bfloat16
AF = mybir.ActivationFunctionType
ALU = mybir.AluOpType

# ---------------- model geometry (hardcoded from the problem spec) ----------
B, IMG, PATCH = 32, 128, 8
D, H, L, MLP, NCLS = 512, 8, 8, 2048, 3
GRID = IMG // PATCH            # 16
P = GRID * GRID                # 256 patches / image
N = P + 1                      # 257 tokens / image
HD = D // H                    # 64
KC = 320                       # im2col contraction: 5 shifts * 8 * 8
NCORES = 8
NIMG = B // NCORES             # 4 images / core
TP = NIMG * P                  # 1024 patch tokens / core
NT = NIMG * N                  # 1028 transformer tokens / core
NTILE = 9                      # token tiles of 128
TT = NTILE * 128               # 1152 padded tokens
IMGOFF = [i * N for i in range(NIMG)]
NEG = -1.0e30
NQ = 258                       # score q width (257 real + 1 pad col)
NQR = 257                      # real q cols
QKW = NT + 1                   # qkt cols (covers the pad col read)
BF_NP = ml_dtypes.bfloat16

_PROGRAM_CACHE = {}

# ============================================================================
# environment fixups
# ============================================================================
_fixups_done = False


def _install_fixups():
    global _fixups_done
    if _fixups_done:
        return
    _fixups_done = True
    MAXW = 1

    def _split_waits(nc, ordered):
        for bb_name, insts in ordered.items():
            new_list = []
            for inst in insts:
                si = getattr(inst, 'sync_info', None)
                eng = getattr(inst, 'engine', None)
                if (si is not None and si.on_wait and len(si.on_wait) > MAXW
                        and eng is not None
                        and type(inst).__name__.startswith('Inst')):
                    waits = list(si.on_wait)
                    inst.sync_info = mybir.SyncInfo(
                        on_wait=waits[:MAXW], on_update=list(si.on_update or []))
                    for i in range(MAXW, len(waits), MAXW):
                        new_list.append(mybir.InstNoOp(
                            name=nc.get_next_instruction_name(),
                            engine=eng, bass_nofuse=True,
                            sync_info=mybir.SyncInfo(
                                on_wait=waits[i:i + MAXW], on_update=[])))
                new_list.append(inst)
            ordered[bb_name] = new_list

    orig_lower = tile.TileContext._lower_ordered_insts

    def patched_lower(self, ordered):
        _split_waits(self.nc, ordered)
        return orig_lower(self, ordered)

    tile.TileContext._lower_ordered_insts = patched_lower

    def patched_drain_and_barrier(self, tick_clock, wait_clock):
        drain_inst = self.nc.sync.drain()
        wait_clock.add_sem_waits(
            drain_inst.ins, ScopedClock({None: tick_clock.global_clock}))
        si = drain_inst.ins.sync_info
        if si and si.on_wait and len(si.on_wait) > MAXW:
            waits = list(si.on_wait)
            drain_inst.ins.sync_info = mybir.SyncInfo(
                on_wait=waits[:MAXW], on_update=list(si.on_update or []))
            for i in range(MAXW, len(waits), MAXW):
                extra = self.nc.sync.drain()
                extra.ins.sync_info = mybir.SyncInfo(
                    on_wait=waits[i:i + MAXW], on_update=[])
        self.nc.all_engine_barrier()
        assert self.sems is not None
        popped = self.nc._tile_sem_poison_stack.pop()
        assert popped is self._sem_poison
        self.nc.clear_and_free_semaphores(list(self.sems.allocated().values()))
        self.nc.all_engine_barrier()

    tile.TileContext._drain_and_barrier = patched_drain_and_barrier

    if 'antenv.axon_hooks' not in sys.modules:
        holder = {'h': None}
        mod = types.ModuleType('antenv.axon_hooks')
        mod.set_axon_ntff_profile_hook = lambda h: holder.__setitem__('h', h)
        mod.get_axon_ntff_profile_hook = lambda: holder['h']
        sys.modules['antenv.axon_hooks'] = mod
        try:
            lib = ctypes.CDLL('/opt/axon/libaxon_pjrt.so')
            if hasattr(lib, 'axon_start_nrt_profile'):
                lib.axon_start_nrt_profile.argtypes = [
                    ctypes.POINTER(ctypes.c_int64), ctypes.c_size_t]
                lib.axon_start_nrt_profile.restype = ctypes.c_int64
                lib.axon_stop_nrt_profile.argtypes = [ctypes.c_char_p]
                lib.axon_stop_nrt_profile.restype = ctypes.c_int64

                @contextlib.contextmanager
                def _hook(output_dir, device_ids):
                    import jax
                    jax.devices()
                    if device_ids:
                        ids = (ctypes.c_int64 * len(device_ids))(*device_ids)
                        rc = lib.axon_start_nrt_profile(ids, len(device_ids))
                    else:
                        rc = lib.axon_start_nrt_profile(None, 0)
                    if rc != 0:
                        raise RuntimeError(f'axon_start_nrt_profile rc={rc}')
                    try:
                        yield
                    finally:
                        lib.axon_stop_nrt_profile(output_dir.encode())

                mod.set_axon_ntff_profile_hook(_hook)
        except OSError:
            pass


# ============================================================================
# host-side input marshaling (pure data movement + tiny param folds)
# ============================================================================
def _im2col(image):
    """(Bc,1,IMG,IMG) -> (Bc, P, 320), col order [shift, py, px]."""
    shifts = [(0, 0), (1, 1), (-1, 1), (1, -1), (-1, -1)]
    x = image[:, 0]
    cols = []
    for (sy, sx) in shifts:
        xs = np.roll(x, (sy, sx), (1, 2))
        pt = xs.reshape(-1, GRID, PATCH, GRID, PATCH).transpose(0, 1, 3, 2, 4)
        cols.append(pt.reshape(-1, P, PATCH * PATCH))
    return np.concatenate(cols, -1)


def _bf(a):
    return np.ascontiguousarray(np.asarray(a, np.float32).astype(BF_NP))


def _host_prep(inputs):
    f = lambda k: np.ascontiguousarray(np.asarray(inputs[k], np.float32))
    image = f('image')

    # conv weights -> matmul form, both tokenizers side by side
    wconv = np.concatenate(
        [f('ssw').reshape(D, KC).T, f('sow').reshape(D, KC).T], 1)  # (320,1024)
    bconv = np.concatenate([f('ssb'), f('sob')])                    # (1024,)
    gbeta = np.stack([np.concatenate([f('ssg'), f('sog')]),
                      np.concatenate([f('ssbeta'), f('sobeta')])])  # (2,1024)

    # fold LN gains/biases into the following matmuls (exact rewrite)
    ln1g, ln1b = f('ln1g'), f('ln1b')
    ln2g, ln2b = f('ln2g'), f('ln2b')
    qkvw, qkvb = f('qkvw'), f('qkvb')
    projw, projb = f('projw'), f('projb')
    w1, b1 = f('w1'), f('b1')
    qkvw_eff = ln1g[:, :, None] * qkvw
    qkvb_eff = qkvb + np.einsum('ld,ldn->ln', ln1b, qkvw)
    w1_eff = ln2g[:, :, None] * w1
    b1_eff = b1 + np.einsum('ld,ldn->ln', ln2b, w1)
    hw_eff = f('ng')[:, None] * f('hw')
    # V bias contributes exactly bv @ projw to the proj output (softmax
    # weights sum to 1 after the 1/denom divide) -> fold into projb
    bv = qkvb_eff[:, 2 * D:3 * D]                      # (L, 512)
    projb_eff = projb + np.einsum('ld,ldn->ln', bv, projw)

    # pos/cls in padded transformer layout
    pos = f('pos_embed')[0]          # (257, 512)
    cls_eff = f('cls_token')[0, 0] + pos[0]
    pospad = np.zeros((TT, D), np.float32)
    for i in range(NIMG):
        pospad[IMGOFF[i]] = cls_eff
        pospad[IMGOFF[i] + 1: IMGOFF[i] + N] = pos[1:]

    X = _im2col(image)               # (B, P, 320)

    common = dict(
        wconv=_bf(wconv), bconv=_bf(bconv), gbeta=_bf(gbeta),
        fw=_bf(f('fw')), fb=_bf(f('fb')), pospad=pospad,
        qkvw=_bf(qkvw_eff),
        qkvbqk=np.ascontiguousarray(qkvb_eff[:, 0:2 * D]),
        projw=_bf(projw), projb=_bf(projb_eff), temp=f('temp'),
        w1=_bf(w1_eff), b1=np.ascontiguousarray(b1_eff),
        w2=_bf(f('w2')), b2=_bf(f('b2')),
        hw=np.ascontiguousarray(
            np.concatenate([hw_eff, np.zeros((D, 1), np.float32)], 1)),
        hb=np.ascontiguousarray(
            np.concatenate([f('hb') + f('nb') @ f('hw'),
                            np.zeros(1, np.float32)])),
    )
    in_maps = []
    for c in range(NCORES):
        # token-layout im2col: col = transformer token index, cls cols zero
        xt = np.zeros((KC, NT), np.float32)
        for i in range(NIMG):
            xt[:, IMGOFF[i] + 1:IMGOFF[i] + N] = X[c * NIMG + i].T
        m = dict(common)
        m['xt'] = _bf(xt)
        in_maps.append(m)
    return in_maps


# ============================================================================
# device program
# ============================================================================
def _tile_segments(t):
    """Real-token segments of token-tile t: (row_in_tile, n, img, pos0)."""
    segs = []
    r0 = 128 * t
    for img in range(NIMG):
        lo = max(r0, IMGOFF[img])
        hi = min(r0 + 128, IMGOFF[img] + N, NT)
        if lo < hi:
            segs.append((lo - r0, hi - lo, img, lo - IMGOFF[img]))
    return segs


def _tile_rows(t):
    """Number of real token rows in token-tile t."""
    return min(128, max(0, NT - 128 * t))


def _build_program():
    nc = bass.Bass()

    din = lambda nm, sh, dt_=F32: nc.dram_tensor(nm, sh, dt_, kind='ExternalInput')
    xt_d = din('xt', [KC, NT], BF16)
    wc_d = din('wconv', [KC, 2 * D], BF16)
    bc_d = din('bconv', [2 * D], BF16)
    gb_d = din('gbeta', [2, 2 * D], BF16)
    fw_d = din('fw', [2 * D, D], BF16)
    fb_d = din('fb', [D], BF16)
    pos_d = din('pospad', [TT, D])
    qkvw_d = din('qkvw', [L, D, 3 * D], BF16)
    qkvbqk_d = din('qkvbqk', [L, 2 * D])
    projw_d = din('projw', [L, D, D], BF16)
    projb_d = din('projb', [L, D], BF16)
    temp_d = din('temp', [L, H])
    w1_d = din('w1', [L, D, MLP], BF16)
    b1_d = din('b1', [L, MLP])
    w2_d = din('w2', [L, MLP, D], BF16)
    b2_d = din('b2', [L, D], BF16)
    hw_d = din('hw', [D, 4])
    hb_d = din('hb', [4])
    out_d = nc.dram_tensor('out', [NIMG, 4], F32, kind='ExternalOutput')

    with tile.TileContext(nc) as tc, \
            nc.allow_low_precision(reason='bf16 matmul operands'):
        with contextlib.ExitStack() as ctx:
            sb = ctx.enter_context(tc.tile_pool(name='sb', bufs=1))
            ps = ctx.enter_context(tc.tile_pool(name='ps', bufs=4, space='PSUM'))
            ps2 = ctx.enter_context(tc.tile_pool(name='ps2', bufs=2, space='PSUM'))

            _psn = [0]

            def pstile(dt_=F32):
                _psn[0] += 1
                return ps.tile([128, 512], dt_, tag='ps', bufs=4,
                               name=f'ps{_psn[0]}')

            # ---------------- constants ----------------
            ident = sb.tile([128, 128], F32, tag='ident')
            make_identity(nc, ident)
            ident_bf = sb.tile([128, 128], BF16, tag='identbf')
            nc.vector.tensor_copy(out=ident_bf, in_=ident)
            negid_f = sb.tile([128, 128], F32, tag='negidf')
            nc.scalar.mul(out=negid_f, in_=ident, mul=NEG)
            c2negr = sb.tile([2, NQ], BF16, tag='c2negr')
            nc.vector.memset(c2negr, NEG)
            nc.vector.memset(c2negr[0:1, :], 0.0)
            nc.vector.memset(c2negr[0:1, 256:257], NEG)
            ones_row = sb.tile([1, 128], BF16, tag='ones_row')
            nc.vector.memset(ones_row, 1.0)
            eps = sb.tile([128, 1], F32, tag='eps')
            nc.vector.memset(eps, 1e-5)

            # ---------------- persistent activations ----------------
            tok = sb.tile([128, NTILE, D], F32, tag='tok')       # residual

            lay = None  # transformer pool; opened after the embed pool closes

            # ---- LN helpers (split so ACT parts can be emission-grouped) ---
            def ln_stats(t):
                stats = lay.tile([128, 6], F32, tag='lnstat', bufs=6)
                mv = lay.tile([128, 2], F32, tag='lnmv', bufs=6)
                nc.vector.bn_stats(out=stats, in_=tok[:, t, :])
                nc.vector.bn_aggr(out=mv, in_=stats)
                return mv

            def ln_rstd(mv, fence=None):
                """Ln/Exp rstd chain (the ACT-table-sensitive part). `fence`
                delays it (via a value-preserving 1-element rewrite of mv)
                so the scheduler cannot pull the Ln/Exp into a gelu stretch
                and thrash the ACT table."""
                if fence is not None:
                    nc.vector.scalar_tensor_tensor(
                        out=mv[0:1, :], in0=mv[0:1, :], scalar=1.0,
                        in1=fence, op0=ALU.mult, op1=ALU.bypass)
                lnv = lay.tile([128, 1], F32, tag='lnv', bufs=10)
                nc.scalar.activation(out=lnv, in_=mv[:, 1:2], func=AF.Ln,
                                     bias=eps, scale=1.0)
                rstd = lay.tile([128, 1], F32, tag='lnrstd', bufs=10)
                nc.scalar.activation(out=rstd, in_=lnv, func=AF.Exp,
                                     scale=-0.5)
                nmr = lay.tile([128, 1], F32, tag='lnnmr', bufs=10)
                nc.vector.scalar_tensor_tensor(
                    out=nmr, in0=mv[:, 0:1], scalar=-1.0,
                    in1=rstd, op0=ALU.mult, op1=ALU.mult)
                return rstd, nmr

            def ln_apply_tr(t, rstd, nmr, xT_dst):
                """Identity apply + transposes + copy (table-neutral)."""
                xn = lay.tile([128, D], BF16, tag='xn', bufs=3)
                nc.scalar.activation(out=xn, in_=tok[:, t, :],
                                     func=AF.Identity, scale=rstd, bias=nmr)
                tp = pstile(BF16)
                for c in range(4):
                    nc.tensor.transpose(tp[:, c * 128:(c + 1) * 128],
                                        xn[:, c * 128:(c + 1) * 128], ident_bf)
                nc.vector.tensor_copy(
                    out=xT_dst[:, :, t * 128:(t + 1) * 128], in_=tp)

            def ln_stats_rstd(t, fence=None):
                return ln_rstd(ln_stats(t), fence=fence)

            # tile groups matching the 512/512/tail column groups that
            # B_qk / MLP1 consume; applies interleave with those groups
            _LN_GROUPS = ((0, 1, 2, 3), (4, 5, 6, 7), (8,))

            def ln_sweep(rn, xT_dst, group_emitters=None):
                for gi, tiles in enumerate(_LN_GROUPS):
                    for t in tiles:
                        ln_apply_tr(t, rn[t][0], rn[t][1], xT_dst)
                    if group_emitters is not None:
                        group_emitters[gi]()

            # ================= patch embed =================
            # the im2col input is already in transformer token layout (cls
            # cols zero), so conv/gate/fuse run per token tile and tokens
            # land in `tok` without a DRAM reshuffle
            with tc.tile_pool(name='emb', bufs=1) as emb:
                xt_sb = []
                for kc, k0, kn in ((0, 0, 128), (1, 128, 128), (2, 256, 64)):
                    t_ = emb.tile([kn, NT], BF16, tag=f'xt{kc}')
                    nc.sync.dma_start(out=t_, in_=xt_d[k0:k0 + kn, :])
                    xt_sb.append(t_)
                wc_sb = []
                for kc, k0, kn in ((0, 0, 128), (1, 128, 128), (2, 256, 64)):
                    t_ = emb.tile([kn, 2 * D], BF16, tag=f'wc{kc}')
                    nc.sync.dma_start(out=t_, in_=wc_d[k0:k0 + kn, :])
                    wc_sb.append(t_)
                bc_sb = emb.tile([1, 2 * D], BF16, tag='bc')
                nc.sync.dma_start(out=bc_sb, in_=bc_d[None, :])
                gb_g = emb.tile([128, 2 * D], BF16, tag='gbg')
                nc.sync.dma_start(
                    out=gb_g, in_=gb_d[0][None, :].to_broadcast([128, 2 * D]))
                gb_b = emb.tile([128, 2 * D], BF16, tag='gbb')
                nc.sync.dma_start(
                    out=gb_b, in_=gb_d[1][None, :].to_broadcast([128, 2 * D]))
                fw_sb = emb.tile([128, 8, D], BF16, tag='fwsb')
                nc.sync.dma_start(
                    out=fw_sb, in_=fw_d[:, :].rearrange('(c p) n -> p c n', p=128))
                fb_sb = emb.tile([1, D], BF16, tag='fbsb')
                nc.sync.dma_start(out=fb_sb, in_=fb_d[None, :])

                # cls token rows per token tile: (tile, row)
                cls_rows = {IMGOFF[i] // 128: IMGOFF[i] % 128
                            for i in range(NIMG)}
                nc.vector.memset(tok[:, NTILE - 1, :], 0.0)
                combs, ggs = [], []
                # sweep 1: conv + LN + gains + transposes + gate matmul
                # (ACT stays in the ln/exp/identity table)
                for t in range(NTILE):
                    rows = _tile_rows(t)
                    c0 = t * 128
                    combraw = emb.tile([128, 2 * D], F32, tag='combraw', bufs=3)
                    for nh in range(2):
                        cps = pstile()
                        nc.tensor.matmul(cps[0:rows, :], ones_row[0:1, 0:rows],
                                         bc_sb[0:1, nh * D:(nh + 1) * D],
                                         start=True, stop=False,
                                         skip_group_check=True)
                        for kc in range(3):
                            nc.tensor.matmul(
                                cps[0:rows, :], xt_sb[kc][:, c0:c0 + rows],
                                wc_sb[kc][:, nh * D:(nh + 1) * D],
                                start=False, stop=(kc == 2),
                                skip_group_check=True)
                        nc.vector.tensor_copy(
                            out=combraw[0:rows, nh * D:(nh + 1) * D],
                            in_=cps[0:rows, :])

                    comb = emb.tile([128, 2 * D], BF16, tag='comb', bufs=9)
                    for nh in range(2):
                        sl = slice(nh * D, (nh + 1) * D)
                        stats = emb.tile([128, 6], F32, tag='estat', bufs=4)
                        mv = emb.tile([128, 2], F32, tag='emv', bufs=4)
                        nc.vector.bn_stats(out=stats[0:rows],
                                           in_=combraw[0:rows, sl])
                        nc.vector.bn_aggr(out=mv[0:rows], in_=stats[0:rows])
                        lnv = emb.tile([128, 1], F32, tag='elnv', bufs=4)
                        nc.scalar.activation(out=lnv[0:rows],
                                             in_=mv[0:rows, 1:2],
                                             func=AF.Ln, bias=eps[0:rows],
                                             scale=1.0)
                        rstd = emb.tile([128, 1], F32, tag='erstd', bufs=4)
                        nc.scalar.activation(out=rstd[0:rows], in_=lnv[0:rows],
                                             func=AF.Exp, scale=-0.5)
                        nmr = emb.tile([128, 1], F32, tag='enmr', bufs=4)
                        nc.vector.scalar_tensor_tensor(
                            out=nmr[0:rows], in0=mv[0:rows, 0:1], scalar=-1.0,
                            in1=rstd[0:rows], op0=ALU.mult, op1=ALU.mult)
                        nc.scalar.activation(out=comb[0:rows, sl],
                                             in_=combraw[0:rows, sl],
                                             func=AF.Identity,
                                             scale=rstd[0:rows],
                                             bias=nmr[0:rows])
                        last_rstd = rstd
                    nc.vector.tensor_mul(comb[0:rows], comb[0:rows], gb_g[0:rows])
                    nc.vector.tensor_add(comb[0:rows], comb[0:rows], gb_b[0:rows])

                    combT = emb.tile([128, 8, 128], BF16, tag='combT', bufs=3)
                    for half in range(2):
                        tp = ps2.tile([128, 2, 512], BF16, tag='stp2', bufs=2,
                                      name=f'etp{t}_{half}')
                        for c in range(4):
                            cc = half * 4 + c
                            nc.tensor.transpose(
                                tp[:, 0, c * rows:(c + 1) * rows],
                                comb[0:rows, cc * 128:(cc + 1) * 128],
                                ident_bf[0:rows, 0:rows])
                        nc.vector.tensor_copy(
                            out=combT[:, half * 4:(half + 1) * 4, 0:rows],
                            in_=tp[:, 0, 0:4 * rows].rearrange(
                                'p (c n) -> p c n', c=4))

                    gps = pstile()
                    nc.tensor.matmul(gps[0:rows, :], ones_row[0:1, 0:rows],
                                     fb_sb, start=True, stop=False,
                                     skip_group_check=True)
                    for c in range(8):
                        nc.tensor.matmul(gps[0:rows, :], combT[:, c, 0:rows],
                                         fw_sb[:, c, :],
                                         start=False, stop=(c == 7),
                                         skip_group_check=True)
                    gg = emb.tile([128, D], F32, tag='gg', bufs=9)
                    nc.scalar.copy(out=gg[0:rows], in_=gps[0:rows, :])
                    combs.append(comb)
                    ggs.append(gg)

                # sweep 2: sigmoids, fenced behind the last embed-LN rstd
                # so the scheduler cannot interleave them with the Ln/Exp
                # chains and thrash the ACT table; then fuse into tok
                for t in range(NTILE):
                    rows = _tile_rows(t)
                    nc.vector.scalar_tensor_tensor(
                        out=ggs[t][0:1, 0:1], in0=ggs[t][0:1, 0:1],
                        scalar=1.0, in1=last_rstd[0:1, 0:1],
                        op0=ALU.mult, op1=ALU.bypass)
                    gt = emb.tile([128, D], F32, tag='gt', bufs=2)
                    nc.scalar.activation(out=gt[0:rows], in_=ggs[t][0:rows],
                                         func=AF.Sigmoid)
                    diff = emb.tile([128, D], F32, tag='diff', bufs=2)
                    nc.vector.tensor_sub(diff[0:rows], combs[t][0:rows, 0:D],
                                         combs[t][0:rows, D:2 * D])
                    nc.vector.tensor_mul(diff[0:rows], diff[0:rows],
                                         gt[0:rows])
                    nc.vector.tensor_add(diff[0:rows], diff[0:rows],
                                         combs[t][0:rows, D:2 * D])
                    postile = emb.tile([128, D], F32, tag='pos', bufs=3)
                    nc.gpsimd.dma_start(out=postile,
                                        in_=pos_d[t * 128:(t + 1) * 128, :])
                    nc.vector.tensor_add(tok[0:rows, t, :], diff[0:rows],
                                         postile[0:rows])
                    if t in cls_rows:
                        # cls rows carry conv-of-zeros junk; overwrite with
                        # cls_eff straight from pospad (DMA may address any
                        # partition, unlike the compute engines)
                        r = cls_rows[t]
                        nc.sync.dma_start(
                            out=tok[r:r + 1, t, :],
                            in_=pos_d[t * 128 + r:t * 128 + r + 1, :])

            # ================= transformer layers =================
            lay = ctx.enter_context(tc.tile_pool(name='lay', bufs=1))

            # after C(img), ot cols for these token tiles are complete
            _D_TILES = {0: (0, 1), 1: (2, 3), 2: (4, 5), 3: (6, 7, 8)}

            _BQK_GROUPS = ((0, 512), (512, 512), (1024, QKW - 1024))

            def bqk_group(qkvw_sb, qkvb_sb, qkt, xT, gi):
                g0, gw = _BQK_GROUPS[gi]

                def emit():
                    for fc in range(8):
                        qp = pstile()
                        for c in range(4):
                            nc.tensor.matmul(
                                qp[:, 0:gw],
                                qkvw_sb[:, c, fc * 128:(fc + 1) * 128],
                                xT[:, c, g0:g0 + gw],
                                start=(c == 0), stop=(c == 3))
                        nc.vector.tensor_scalar_add(
                            qkt[:, fc, g0:g0 + gw], qp[:, 0:gw],
                            qkvb_sb[:, fc:fc + 1])
                return emit

            def attn_proj_phase(l, xT, qkt, qkvw_sb, fillers=None):
                """B_v + C + proj(+residual) for layer l. The previous
                layer's deferred MLP2 chunks + this layer's LN1/B_qk arrive
                as `fillers` and are woven between head loops so the dense
                MLP2 streams keep the PE array busy (HAM stays at full
                clock) through the attention phase. Returns LN2 rstd chains
                per tile."""
                temp_sb = lay.tile([128, H], F32, tag='temp', bufs=2)
                nc.sync.dma_start(out=temp_sb,
                                  in_=temp_d[l][None, :].to_broadcast([128, H]))
                projw_sb = lay.tile([128, 4, D], BF16, tag='wt', bufs=3)
                nc.sync.dma_start(
                    out=projw_sb,
                    in_=projw_d[l].rearrange('(c p) n -> p c n', p=128))
                projb_bc = lay.tile([128, D], BF16, tag='projb', bufs=2)
                nc.sync.dma_start(out=projb_bc,
                                  in_=projb_d[l][None, :].to_broadcast([128, D]))

                # ---- B_v + C + proj, interleaved per image ----
                vimg = lay.tile([128, NIMG, 2, H * 65], BF16, tag='vimg',
                                bufs=1)
                nc.gpsimd.memset(
                    vimg.rearrange('p i c (h e) -> p i c h e', e=65)
                    [:, :, :, :, 64:65], 1.0)
                vl = lay.tile([2, NIMG, H * 65], BF16, tag='vlast', bufs=1)
                nc.gpsimd.memset(vl, 0.0)
                nc.gpsimd.memset(
                    vl.rearrange('p i (h e) -> p i h e', e=65)
                    [:, :, :, 64:65], 1.0)
                ot = lay.tile([128, 4, NT], BF16, tag='ot', bufs=2)
                rn2 = {}

                def emit_bv(img):
                    io = IMGOFF[img]
                    for c2 in range(2):
                        vp = pstile()
                        for c in range(4):
                            nc.tensor.matmul(
                                vp, xT[:, c, io + c2 * 128:io + (c2 + 1) * 128],
                                qkvw_sb[:, c, 2 * D:3 * D],
                                start=(c == 0), stop=(c == 3))
                        nc.vector.tensor_copy(
                            out=vimg.rearrange(
                                'p i c (h e) -> p i c h e', e=65)
                            [:, img, c2, :, 0:64],
                            in_=vp)
                    vp2 = pstile()
                    for c in range(4):
                        nc.tensor.matmul(vp2[0:1, :],
                                         xT[:, c, io + P:io + P + 1],
                                         qkvw_sb[:, c, 2 * D:3 * D],
                                         start=(c == 0), stop=(c == 3))
                    nc.scalar.copy(
                        out=vl.rearrange('p i (h e) -> p i h e', e=65)
                        [0:1, img, :, 0:64],
                        in_=vp2[0:1, :])

                def emit_proj(t):
                    # bias rides a Pool-engine add (the attention stretch is
                    # HAM-throttled, so every matmul here costs double)
                    rows = _tile_rows(t)
                    pp = pstile()
                    for c in range(4):
                        nc.tensor.matmul(
                            pp[0:rows, :],
                            ot[:, c, t * 128:t * 128 + rows],
                            projw_sb[:, c, :],
                            start=(c == 0), stop=(c == 3),
                            skip_group_check=True)
                    nc.vector.tensor_add(tok[0:rows, t, :],
                                         tok[0:rows, t, :], pp[0:rows, :])
                    nc.gpsimd.tensor_add(tok[0:rows, t, :],
                                         tok[0:rows, t, :],
                                         projb_bc[0:rows, :])
                    rn2[t] = ln_stats_rstd(t)

                emit_bv(0)
                for img in range(NIMG):
                    io = IMGOFF[img]
                    # attention heads; the next image's fat V matmuls and the
                    # previous image's proj tiles interleave mid-loop to keep
                    # the PE array dense (HAM activity windows)
                    for h in range(H):
                        if h == 4:
                            if img + 1 < NIMG:
                                emit_bv(img + 1)
                            if img >= 1:
                                emit_proj(2 * (img - 1))
                                emit_proj(2 * (img - 1) + 1)
                        hr = (h % 2) * 64
                        qfc = h // 2
                        kfc = 4 + h // 2
                        s2 = ps2.tile([128, 2, 512], F32, tag='stp2', bufs=2)
                        pv = pstile()
                        for c in range(2):
                            nc.tensor.matmul(
                                s2[:, c, 0:NQ],
                                qkt[hr:hr + 64, kfc, io + c * 128:io + (c + 1) * 128],
                                qkt[hr:hr + 64, qfc, io:io + NQ],
                                start=True, stop=True, skip_group_check=True)
                            nc.vector.tensor_add(
                                s2[:, c, c * 128:(c + 1) * 128],
                                s2[:, c, c * 128:(c + 1) * 128], negid_f)
                        # tail-chunk scores borrow pv rows 32:34; the pv
                        # group overwrites them after their exp is consumed
                        nc.tensor.matmul(
                            pv[32:34, 0:NQ],
                            qkt[hr:hr + 64, kfc, io + 256:io + 258],
                            qkt[hr:hr + 64, qfc, io:io + NQ],
                            start=True, stop=False, skip_group_check=True)
                        nc.tensor.matmul(
                            pv[32:34, 0:NQ], ident_bf[0:2, 0:2], c2negr,
                            start=False, stop=True, skip_group_check=True)
                        pt = lay.tile([128, 3, NQ], BF16, tag='pt', bufs=3)
                        nc.scalar.activation(
                            out=pt[:, 0:2, :], in_=s2[:, :, 0:NQ],
                            func=AF.Exp, scale=temp_sb[:, h:h + 1])
                        nc.scalar.activation(
                            out=pt[0:2, 2, :], in_=pv[32:34, 0:NQ],
                            func=AF.Exp, scale=temp_sb[32:34, h:h + 1])
                        for c in range(3):
                            cm = (128, 128, 2)[c]
                            lhs = (vimg[0:cm, img, c, h * 65:h * 65 + 65]
                                   if c < 2
                                   else vl[0:2, img, h * 65:h * 65 + 65])
                            nc.tensor.matmul(
                                pv[0:65, 0:NQ], lhs, pt[0:cm, c, :],
                                start=(c == 0), stop=(c == 2),
                                skip_group_check=True)
                        rr = lay.tile([1, NQ], BF16, tag='rr', bufs=4)
                        nc.vector.reciprocal(out=rr[0:1, 0:NQR],
                                             in_=pv[64:65, 0:NQR])
                        # broadcast 1/denom into pv rows 64:128 (the denom
                        # row is dead once the reciprocal has read it), then
                        # stage it in SBUF (DVE may read only one PSUM input)
                        nc.tensor.matmul(pv[64:128, 0:NQR],
                                         ones_fr[0:1, 0:64],
                                         rr[0:1, 0:NQR].bitcast(F32R),
                                         start=True, stop=True,
                                         skip_group_check=True)
                        rts = lay.tile([64, NQ], BF16, tag='rts', bufs=4)
                        nc.scalar.copy(out=rts[0:64, 0:NQR],
                                       in_=pv[64:128, 0:NQR])
                        nc.vector.tensor_mul(ot[hr:hr + 64, qfc, io:io + NQR],
                                             pv[0:64, 0:NQR],
                                             rts[0:64, 0:NQR])
                # trailing proj tiles (last image's columns)
                for t in (6, 7, 8):
                    emit_proj(t)
                return rn2

            def mlp_phase(l, xT2, ln2_rn, want_next):
                """LN2-apply + MLP(+residual) for layer l, with next layer's
                LN1 applies + B_qk fused into the MLP2 loop per tile group.
                Returns (xT_next, qkt_next, qkvw_next, ln1_rn_next)."""
                w1_sb = lay.tile([128, 4, MLP], BF16, tag='wt', bufs=3)
                nc.sync.dma_start(
                    out=w1_sb, in_=w1_d[l].rearrange('(c p) n -> p c n', p=128))
                b1_sb = lay.tile([128, 16], F32, tag='b1', bufs=2)
                nc.sync.dma_start(
                    out=b1_sb, in_=b1_d[l].rearrange('(c p) -> p c', p=128))
                w2_sb = lay.tile([128, 16, D], BF16, tag='wt', bufs=3)
                nc.sync.dma_start(
                    out=w2_sb, in_=w2_d[l].rearrange('(c p) n -> p c n', p=128))
                b2_sb = lay.tile([1, D], BF16, tag='b2', bufs=2)
                nc.sync.dma_start(out=b2_sb, in_=b2_d[l][None, :])
                if want_next:
                    qkvw_n = lay.tile([128, 4, 3 * D], BF16, tag='wt', bufs=3)
                    nc.sync.dma_start(
                        out=qkvw_n,
                        in_=qkvw_d[l + 1].rearrange('(c p) n -> p c n', p=128))
                    qkvb_n = lay.tile([128, 8], F32, tag='qkvb', bufs=2)
                    nc.sync.dma_start(
                        out=qkvb_n,
                        in_=qkvbqk_d[l + 1].rearrange('(c p) -> p c', p=128))
                    xT_n = lay.tile([128, 4, TT], BF16, tag='xT', bufs=2)
                    qkt_n = lay.tile([128, 8, QKW], BF16, tag='qkt', bufs=2)
                else:
                    qkvw_n = qkvb_n = xT_n = qkt_n = None

                groups = ((0, 512), (512, 512), (1024, NT - 1024))
                hTs = {}

                def f_group(gi, g0, gw):
                    def emit():
                        hT = lay.tile([128, 16, gw], BF16,
                                      tag=('hT' if gw == 512 else 'hTs'),
                                      bufs=2, name=f'hT{l}_{gi}')
                        hTs[gi] = hT
                        for hc in range(16):
                            hp = pstile()
                            for c in range(4):
                                nc.tensor.matmul(
                                    hp[:, 0:gw],
                                    w1_sb[:, c, hc * 128:(hc + 1) * 128],
                                    xT2[:, c, g0:g0 + gw],
                                    start=(c == 0), stop=(c == 3))
                            nc.scalar.activation(
                                out=hT[:, hc, :], in_=hp[:, 0:gw],
                                func=AF.Gelu, bias=b1_sb[:, hc:hc + 1],
                                scale=1.0)
                    return emit

                ln_sweep(ln2_rn, xT2,
                         [f_group(gi, g0, gw)
                          for gi, (g0, gw) in enumerate(groups)])

                rn_next = {}
                mvs_next = {}
                for gi, (g0, gw) in enumerate(groups):
                    hT = hTs[gi]
                    ntr = (gw + 127) // 128
                    for tr in range(ntr):
                        t = (g0 + tr * 128) // 128
                        rows = min(128, gw - tr * 128)
                        mp = pstile()
                        nc.tensor.matmul(mp[0:rows, :], ones_row[0:1, 0:rows],
                                         b2_sb, start=True, stop=False,
                                         skip_group_check=True)
                        for c in range(16):
                            nc.tensor.matmul(
                                mp[0:rows, :],
                                hT[:, c, tr * 128:tr * 128 + rows],
                                w2_sb[:, c, :],
                                start=False, stop=(c == 15),
                                skip_group_check=True)
                        nc.vector.tensor_add(tok[0:rows, t, :],
                                             tok[0:rows, t, :], mp[0:rows, :])
                        if want_next:
                            mvs_next[t] = ln_stats(t)
                if want_next:
                    gelu_fence = hTs[3][:, 15, 0:2]
                    for t in range(NTILE):
                        rn_next[t] = ln_rstd(mvs_next[t], fence=gelu_fence)
                    ln_sweep(rn_next, xT_n,
                             [bqk_group(qkvw_n, qkvb_n, qkt_n, xT_n, gi)
                              for gi in range(4)])
                return xT_n, qkt_n, qkvw_n, rn_next

            # layer 0 prologue: LN1 + B_qk
            rn1 = {t: ln_stats_rstd(t) for t in range(NTILE)}
            qkvw_sb = lay.tile([128, 4, 3 * D], BF16, tag='wt', bufs=3)
            nc.sync.dma_start(
                out=qkvw_sb,
                in_=qkvw_d[0].rearrange('(c p) n -> p c n', p=128))
            qkvb_sb = lay.tile([128, 8], F32, tag='qkvb', bufs=2)
            nc.sync.dma_start(
                out=qkvb_sb,
                in_=qkvbqk_d[0].rearrange('(c p) -> p c', p=128))
            xT = lay.tile([128, 4, TT], BF16, tag='xT', bufs=2)
            qkt = lay.tile([128, 8, QKW], BF16, tag='qkt', bufs=2)
            ln_sweep(rn1, xT,
                     [bqk_group(qkvw_sb, qkvb_sb, qkt, xT, gi)
                      for gi in range(4)])
            for l in range(L):
                rn2 = attn_proj_phase(l, xT, qkt, qkvw_sb)
                xT2 = lay.tile([128, 4, TT], BF16, tag='xT', bufs=2)
                xT, qkt, qkvw_sb, rn1 = mlp_phase(l, xT2, rn2, l < L - 1)

            # ================= head =================
            hw_sb = lay.tile([128, 4, 4], F32, tag='hwsb')
            nc.sync.dma_start(out=hw_sb,
                              in_=hw_d[:, :].rearrange('(c p) n -> p c n', p=128))
            hb_sb = lay.tile([1, 4], F32, tag='hbsb')
            nc.sync.dma_start(out=hb_sb, in_=hb_d[None, :])

            cls_sb = lay.tile([NIMG, D], F32, tag='cls')
            for img in range(NIMG):
                r = IMGOFF[img]
                nc.sync.dma_start(out=cls_sb[img:img + 1, :],
                                  in_=tok[r % 128:r % 128 + 1, r // 128, :])
            # final LN on the 4 cls tokens
            stats = lay.tile([NIMG, 6], F32, tag='hstat')
            mv = lay.tile([NIMG, 2], F32, tag='hmv')
            nc.vector.bn_stats(out=stats, in_=cls_sb[0:NIMG, :])
            nc.vector.bn_aggr(out=mv, in_=stats)
            lnv = lay.tile([NIMG, 1], F32, tag='hlnv')
            nc.scalar.activation(out=lnv, in_=mv[:, 1:2], func=AF.Ln,
                                 bias=eps[0:NIMG], scale=1.0)
            rstd = lay.tile([NIMG, 1], F32, tag='hrstd')
            nc.scalar.activation(out=rstd, in_=lnv, func=AF.Exp, scale=-0.5)
            nmr = lay.tile([NIMG, 1], F32, tag='hnmr')
            nc.vector.scalar_tensor_tensor(
                out=nmr, in0=mv[:, 0:1], scalar=-1.0,
                in1=rstd, op0=ALU.mult, op1=ALU.mult)
            clsn = lay.tile([NIMG, D], F32, tag='clsn')
            nc.scalar.activation(out=clsn, in_=cls_sb[0:NIMG, :],
                                 func=AF.Identity, scale=rstd, bias=nmr)
            clsT = lay.tile([128, 4, NIMG], F32, tag='clsT')
            for c in range(4):
                tp = pstile()
                nc.tensor.transpose(tp[0:128, 0:NIMG],
                                    clsn[0:NIMG, c * 128:(c + 1) * 128],
                                    ident[0:NIMG, 0:NIMG])
                nc.vector.tensor_copy(out=clsT[:, c, :], in_=tp[0:128, 0:NIMG])
            onesf = lay.tile([1, NIMG], F32, tag='onesf')
            nc.vector.memset(onesf, 1.0)
            op = pstile()
            nc.tensor.matmul(op[0:NIMG, 0:4], onesf[0:1, 0:NIMG], hb_sb,
                             start=True, stop=False, skip_group_check=True)
            for c in range(4):
                nc.tensor.matmul(op[0:NIMG, 0:4], clsT[:, c, :],
                                 hw_sb[:, c, :],
                                 start=False, stop=(c == 3),
                                 skip_group_check=True)
            osb = lay.tile([NIMG, 4], F32, tag='osb')
            nc.vector.tensor_copy(out=osb[0:NIMG, :], in_=op[0:NIMG, 0:4])
            nc.sync.dma_start(out=out_d[:, :], in_=osb[0:NIMG, :])

    return nc


# ============================================================================
# entry point
# ============================================================================
def kernel(**inputs) -> np.ndarray:
    _install_fixups()
    from concourse.bass_utils import run_bass_kernel_spmd

    if 'nc' not in _PROGRAM_CACHE:
        _PROGRAM_CACHE['nc'] = _build_program()
    nc = _PROGRAM_CACHE['nc']

    in_maps = _host_prep(inputs)
    res = run_bass_kernel_spmd(nc, in_maps, core_ids=list(range(NCORES)))
    out = np.concatenate([np.asarray(res.results[i]['out'])
                          for i in range(NCORES)], 0)
    return out[:, :NCLS].astype(np.float32)
